# revision 13
# baseline (speedup 1.0000x reference)
"""GAT policy network (3-layer GAT + global mean pool head) on 8 Trainium2
NeuronCores via Bass/Tile.

Sharding: nodes are dealt to the 8 cores (graph/data parallel); each core owns
6250 dst nodes (padded to 6272 = 49 tiles x 128) and all edges incident on
them by destination.  Small GAT weights are replicated.

v2 design:
  * Table rows are [128 x fp8 xw | 4 x bf16 sc_s] = 136B payload at 256B
    stride, AllGather-replicated per layer; per-edge rows fetched with
    dma_gather through two int16-index windows (A: rows [0,32768), B: rows
    [17408, 50176)).
  * Chunk-uniform slot grid: the 49 dst tiles are partitioned into chunks of
    <= TCMAX tiles; within a chunk every tile gets the same A-slot and B-slot
    counts, so alpha / leaky-relu / exp / denominators batch into a handful of
    wide DVE/Act instructions per chunk instead of per-tile ops.
  * Self-loop scores use sc_e_loop = mean_attr @ (We a_e), precomputed on the
    host per layer -> the whole self-loop chain is a few batched ops.
  * Message aggregation: per tile, scl = gathered_xw * exp(alpha) (DVE), then
    either identity-matmul accumulation on PE or a strided tensor_reduce on
    DVE (split tunable to balance engines).
"""

import sys
sys.path.insert(0, '/opt/trn_rl_repo')

import inspect
import textwrap

import numpy as np
import ml_dtypes

import concourse.bass as bass
import concourse.bacc as bacc
import concourse.tile as tile
import concourse.mybir as mybir

bf16 = ml_dtypes.bfloat16
fp8 = ml_dtypes.float8_e4m3
F32 = mybir.dt.float32
BF16 = mybir.dt.bfloat16
F8 = mybir.dt.float8e4
I16 = mybir.dt.int16

# problem dims
N, E, F_IN, ED = 50000, 800000, 64, 16
H, C = 4, 32
HC = H * C
B, A = 64, 8
NEG_SLOPE = 0.2
NCORE = 8
NLOC = 6272
NT = 49
NTOT = NCORE * NLOC          # 50176
STRIDE = 256                 # table row stride, bytes (f8 elems)
ROWB = 136                   # gathered payload bytes per row
WINB = 17408
WINA_MAX = 32767
POISON = -1.0e38
POISON_A = 6271              # abs row, inside window A (padding row core 0)
POISON_B = 3 * NLOC + 6271   # abs row 25087, inside window B (padding row core 3)
TCMAX = 4                    # max tiles per chunk
ACT_CAST_MOD7 = 5            # of every 7 tiles, this many use the Act-cast 2x path


def _patch_dma_gather():
    """Relax the elem_size_bytes % 256 assert (transpose-only restriction; the
    non-transpose HBM path takes arbitrary payload length, only the row stride
    must be a multiple of 256B)."""
    if getattr(bass.BassGpSimd.dma_gather, "_gat_patched", False):
        return
    src = textwrap.dedent(inspect.getsource(bass.BassGpSimd.dma_gather))
    needle = (
        "    assert (\n"
        "        elem_size_bytes > 0 and elem_size_bytes % 256 == 0\n"
        "    )  # transpose restriction\n"
    )
    assert needle in src, "dma_gather source changed; patch needs update"
    src = src.replace(
        needle,
        "    assert elem_size_bytes > 0\n"
        "    if transpose:\n"
        "        assert elem_size_bytes % 256 == 0\n",
    )
    ns = vars(bass).copy()
    exec(compile(src, "<patched dma_gather>", "exec"), ns)
    fn = ns["dma_gather"]
    fn._gat_patched = True
    bass.BassGpSimd.dma_gather = fn


# ===================================================================== prep
def _prep(inputs):
    x = np.asarray(inputs["x"], np.float32)
    edge_attr = np.asarray(inputs["edge_attr"], np.float32)
    edge_index = np.asarray(inputs["edge_index"]).astype(np.int64)
    batch = np.asarray(inputs["batch"]).astype(np.int64)
    src, dst = edge_index[0], edge_index[1]

    deg = np.bincount(dst, minlength=N)
    odeg = np.bincount(src, minlength=N)

    # node -> core; put high out-degree nodes into cores whose table blocks
    # fall in the shared window region (cores 3,4,2,5 cover rows ~12.5K-37.6K)
    order = np.argsort(-odeg, kind="stable")
    owner = np.empty(N, np.int64)
    for i, c in enumerate([3, 4, 2, 5, 1, 6, 0, 7]):
        owner[order[i * 6250:(i + 1) * 6250]] = c

    local = np.empty(N, np.int64)
    nodes_of = []
    for c in range(NCORE):
        mine = np.where(owner == c)[0]
        mine = mine[np.argsort(-deg[mine], kind="stable")]
        local[mine] = np.arange(6250)
        nodes_of.append(mine)
    pos = owner * NLOC + local

    # --- per-core, per-node edge lists with A-only / B-only / free counts ---
    ecore = owner[dst]
    eloc = local[dst]
    # per (core, local node): lists of edges
    edges_of = [None] * NCORE
    nAonly = np.zeros((NCORE, NLOC), np.int64)
    nBonly = np.zeros((NCORE, NLOC), np.int64)
    ndeg = np.zeros((NCORE, NLOC), np.int64)
    for c in range(NCORE):
        sel = np.where(ecore == c)[0]
        d_loc = eloc[sel]
        ord2 = np.argsort(d_loc, kind="stable")
        sel = sel[ord2]
        d_loc = d_loc[ord2]
        spos = pos[src[sel]]
        okA = spos <= WINA_MAX
        okB = spos >= WINB
        bounds = np.searchsorted(d_loc, np.arange(NLOC + 1))
        edges_of[c] = (sel, bounds, okA, okB)
        cntA = np.zeros(NLOC, np.int64)
        cntB = np.zeros(NLOC, np.int64)
        cntD = np.zeros(NLOC, np.int64)
        np.add.at(cntA, d_loc[okA & ~okB], 1)
        np.add.at(cntB, d_loc[okB & ~okA], 1)
        np.add.at(cntD, d_loc, 1)
        nAonly[c] = cntA
        nBonly[c] = cntB
        ndeg[c] = cntD

    # per-tile global lower bounds
    LA = nAonly.reshape(NCORE, NT, 128).max(axis=(0, 2))
    LB = nBonly.reshape(NCORE, NT, 128).max(axis=(0, 2))
    LD = ndeg.reshape(NCORE, NT, 128).max(axis=(0, 2))

    # --- chunk the 49 tiles (DP, chunk size <= TCMAX, minimize padded slots)
    INF = 1 << 60
    CHUNK_COST = 56          # fixed per-chunk cost in slot-column equivalents
    best = [INF] * (NT + 1)
    prev = [0] * (NT + 1)
    best[0] = 0
    for j in range(1, NT + 1):
        for i in range(max(0, j - TCMAX), j):
            la = int(LA[i:j].max())
            lb = int(LB[i:j].max())
            ld = int(LD[i:j].max())
            w = max(ld, la + lb)
            cost = best[i] + (j - i) * w + CHUNK_COST
            if cost < best[j]:
                best[j] = cost
                prev[j] = i
    cuts = []
    j = NT
    while j > 0:
        cuts.append((prev[j], j))
        j = prev[j]
    cuts.reverse()

    chunks = []           # (tiles list, RA, RB, colA0, colB0)
    w = 0
    for (i, j) in cuts:
        la = int(LA[i:j].max())
        lb = int(LB[i:j].max())
        ld = int(LD[i:j].max())
        tot = max(ld, la + lb, 1)
        ra = la + (tot - la - lb + 1) // 2
        ra = max(ra, 1)
        rb = max(tot - ra, 1)
        tiles = list(range(i, j))
        colA0 = w
        colB0 = w + len(tiles) * ra
        w = colB0 + len(tiles) * rb
        chunks.append(dict(tiles=tiles, RA=ra, RB=rb, colA0=colA0, colB0=colB0))
    WTOT = w
    W8 = (WTOT + 7) // 8

    layout = dict(chunks=chunks, WTOT=WTOT, W8=W8)

    gcnt = np.bincount(batch, minlength=B).astype(np.float32)
    cnt = np.maximum(deg, 1.0)
    attr_sum = np.zeros((N, ED), np.float32)
    np.add.at(attr_sum, dst, edge_attr)
    mean_attr = attr_sum / cnt[:, None]

    # per-layer Q = We_l reshaped @ a_e_l  -> [ED, H]
    Qs = []
    for li in (1, 2, 3):
        Wem = np.asarray(inputs[f"We{li}"], np.float32)
        a_e = np.asarray(inputs[f"ae{li}"], np.float32)
        Q = np.zeros((ED, H), np.float32)
        for h in range(H):
            Q[:, h] = Wem[:, h * C:(h + 1) * C] @ a_e[h]
        Qs.append(Q)

    in_maps = []
    for c in range(NCORE):
        sel, bounds, okA, okB = edges_of[c]
        gidx = np.zeros((128, WTOT), np.int64)
        attr_rect = np.zeros((128, W8 * 8, ED), np.float32)
        for ch in chunks:
            ra, rb = ch["RA"], ch["RB"]
            for tl, t in enumerate(ch["tiles"]):
                ca = ch["colA0"] + tl * ra
                cb = ch["colB0"] + tl * rb
                for j in range(128):
                    ln = t * 128 + j
                    lo, hi = bounds[ln], bounds[ln + 1]
                    ea, eb = [], []
                    if lo < hi:
                        free = []
                        for k in range(lo, hi):
                            if okA[k] and okB[k]:
                                free.append(sel[k])
                            elif okA[k]:
                                ea.append(sel[k])
                            else:
                                eb.append(sel[k])
                        for e in free:
                            if len(ea) < ra and (len(ea) - nAonly[c][ln]
                                                 <= len(eb) - nBonly[c][ln]
                                                 or len(eb) >= rb):
                                ea.append(e)
                            else:
                                eb.append(e)
                    assert len(ea) <= ra and len(eb) <= rb, (c, t, j, len(ea), len(eb), ra, rb)
                    for r in range(ra):
                        if r < len(ea):
                            gidx[j, ca + r] = pos[src[ea[r]]]
                            attr_rect[j, ca + r] = edge_attr[ea[r]]
                        else:
                            gidx[j, ca + r] = POISON_A
                    for r in range(rb):
                        if r < len(eb):
                            gidx[j, cb + r] = pos[src[eb[r]]] - WINB
                            attr_rect[j, cb + r] = edge_attr[eb[r]]
                        else:
                            gidx[j, cb + r] = POISON_B - WINB
        assert 0 <= gidx.min() and gidx.max() <= 32767
        gidx = gidx.astype(np.int16)

        # wrapped idx layout: position i=(col-c0)*128+j -> idx16[j%16, col*8+j//16]
        jj = np.arange(128)
        gidxw = np.zeros((16, WTOT * 8), np.int16)
        cols8 = (np.arange(WTOT)[None, :] * 8 + (jj // 16)[:, None])  # [128, WTOT]
        gidxw[(jj % 16)[:, None], cols8] = gidx
        gidxw = np.tile(gidxw, (8, 1))

        # attr8[g, wj*ED+cc, j] = attr_rect[j, 8g+wj, cc]
        a4 = attr_rect.reshape(128, W8, 8, ED)
        attr8 = np.ascontiguousarray(
            a4.transpose(1, 2, 3, 0).reshape(W8, 8 * ED, 128)).astype(bf16)

        xblk = np.zeros((128, NT, F_IN), np.float32)
        pmat = np.zeros((128, NT, B), np.float32)
        msce = np.zeros((128, NT, 12), np.float32)
        mine = nodes_of[c]
        for ln in range(6250):
            t, j = ln // 128, ln % 128
            n = mine[ln]
            xblk[j, t] = x[n]
            pmat[j, t, batch[n]] = 1.0 / max(gcnt[batch[n]], 1.0)
            for li in range(3):
                msce[j, t, li * 4:(li + 1) * 4] = mean_attr[n] @ Qs[li]

        in_maps.append({"gidx": gidxw, "attr8": attr8,
                        "xblk": xblk.astype(bf16),
                        "pmat": pmat.astype(bf16), "msce": msce})

    # weights (replicated)
    wts = {}
    q8s = []
    for li, (Wk, ask, adk, bk) in enumerate(
            [("W1", "as1", "ad1", "b1"),
             ("W2", "as2", "ad2", "b2"),
             ("W3", "as3", "ad3", "b3")]):
        Wm = np.asarray(inputs[Wk], np.float32)
        a_s = np.asarray(inputs[ask], np.float32)
        a_d = np.asarray(inputs[adk], np.float32)
        bv = np.asarray(inputs[bk], np.float32)
        wts[f"w{li+1}"] = Wm.astype(bf16)
        asdb = np.zeros((HC, 8), np.float32)
        for h in range(H):
            asdb[h * C:(h + 1) * C, h] = a_s[h]
            asdb[h * C:(h + 1) * C, 4 + h] = a_d[h]
        wts[f"asdb{li+1}"] = asdb.astype(bf16)
        q8 = np.zeros((128, 32), np.float32)
        for wj in range(8):
            q8[wj * ED:(wj + 1) * ED, wj * 4:(wj + 1) * 4] = Qs[li]
        q8s.append(q8)
        wts[f"bias{li+1}"] = bv.reshape(1, HC)
    wts["qblk"] = np.concatenate(q8s, axis=1).astype(bf16)
    wts["wl"] = np.asarray(inputs["Wl"], np.float32)
    wts["blv"] = np.asarray(inputs["bl"], np.float32).reshape(A, 1)
    wts["ident"] = np.eye(128, dtype=np.float32)
    # poison block: full 256B rows for padding rows 6250..6271:
    # xw bytes = 0, sc_s bytes (128..136) = bf16(-1e38) x4
    pois = np.zeros((NLOC - 6250, 256), np.uint8)
    pb = np.full((4,), POISON, bf16).view(np.uint8)
    pois[:, 128:136] = np.tile(pb, (NLOC - 6250, 1))
    wts["poisblk"] = pois.view(bf16)
    for m in in_maps:
        m.update(wts)
    return in_maps, layout


# ==================================================================== build
def build(layout):
    _patch_dma_gather()
    chunks = layout["chunks"]
    WTOT, W8 = layout["WTOT"], layout["W8"]
    GW = max(len(ch["tiles"]) * (ch["RA"] + ch["RB"]) for ch in chunks)
    RMAX = max(ch["RA"] + ch["RB"] for ch in chunks)

    nc = bacc.Bacc("TRN2", target_bir_lowering=False, debug=False,
                   num_devices=NCORE)

    gidx_in = nc.dram_tensor("gidx", [128, WTOT * 8], I16, kind="ExternalInput")
    attr8_in = nc.dram_tensor("attr8", [W8, 128, 128], BF16, kind="ExternalInput")
    xblk_in = nc.dram_tensor("xblk", [128, NT, F_IN], BF16, kind="ExternalInput")
    pmat_in = nc.dram_tensor("pmat", [128, NT, B], BF16, kind="ExternalInput")
    msce_in = nc.dram_tensor("msce", [128, NT, 12], F32, kind="ExternalInput")
    w_in = {1: nc.dram_tensor("w1", [F_IN, HC], BF16, kind="ExternalInput"),
            2: nc.dram_tensor("w2", [HC, HC], BF16, kind="ExternalInput"),
            3: nc.dram_tensor("w3", [HC, HC], BF16, kind="ExternalInput")}
    asdb_in = {l: nc.dram_tensor(f"asdb{l}", [HC, 8], BF16, kind="ExternalInput")
               for l in (1, 2, 3)}
    bias_in = {l: nc.dram_tensor(f"bias{l}", [1, HC], F32, kind="ExternalInput")
               for l in (1, 2, 3)}
    qblk_in = nc.dram_tensor("qblk", [128, 96], BF16, kind="ExternalInput")
    wl_in = nc.dram_tensor("wl", [HC, A], F32, kind="ExternalInput")
    blv_in = nc.dram_tensor("blv", [A, 1], F32, kind="ExternalInput")
    ident_in = nc.dram_tensor("ident", [128, 128], F32, kind="ExternalInput")
    pois_in = nc.dram_tensor("poisblk", [NLOC - 6250, 128], BF16, kind="ExternalInput")
    out_t = nc.dram_tensor("out", [A, B], F32, kind="ExternalOutput")

    blk = nc.dram_tensor("blk", [NLOC, STRIDE // 2], BF16)
    tblS = nc.dram_tensor("tblS", [NTOT, STRIDE // 2], BF16, addr_space="Shared")
    pool_in = nc.dram_tensor("pool_in", [HC, B], F32)
    pool_sh = nc.dram_tensor("pool_sh", [HC, B], F32, addr_space="Shared")

    tblap = tblS.ap().bitcast(F8)
    winA = tblap[:, :ROWB]
    winB = tblap[WINB:, :ROWB]
    rg = [list(range(NCORE))]

    with tile.TileContext(nc) as tc:
        with (
            tc.tile_pool(name="const", bufs=1) as cpool,
            tc.tile_pool(name="sb", bufs=3) as sb,
            tc.tile_pool(name="sclp", bufs=2) as sclp,
            tc.tile_pool(name="gp", bufs=2) as gp,
            tc.tile_pool(name="pap", bufs=2) as pap,
            tc.tile_pool(name="np2", bufs=2) as np2,
            tc.tile_pool(name="np1", bufs=1) as np1,
            tc.tile_pool(name="psA", bufs=2, space="PSUM") as psA,
            tc.tile_pool(name="psB", bufs=2, space="PSUM") as psB,
            tc.tile_pool(name="psC", bufs=2, space="PSUM") as psC,
            tc.tile_pool(name="psD", bufs=1, space="PSUM") as psD,
            tc.tile_pool(name="psE", bufs=1, space="PSUM") as psE,
        ):
            identf = cpool.tile([128, 128], F32)
            nc.sync.dma_start(identf[:], ident_in.ap())
            identb = cpool.tile([128, 128], BF16)
            nc.vector.tensor_copy(identb[:], identf[:])
            gidx = cpool.tile([128, WTOT * 8], I16)
            nc.sync.dma_start(gidx[:], gidx_in.ap())
            qblk = cpool.tile([128, 96], BF16)
            nc.sync.dma_start(qblk[:], qblk_in.ap())
            xblk = cpool.tile([128, NT, F_IN], BF16)
            nc.sync.dma_start(xblk[:], xblk_in.ap())
            msce = cpool.tile([128, NT, 12], F32)
            nc.sync.dma_start(msce[:], msce_in.ap())
            wts = {}
            for l in (1, 2, 3):
                wt = cpool.tile([F_IN if l == 1 else HC, HC], BF16, tag=f"w{l}")
                nc.sync.dma_start(wt[:], w_in[l].ap())
                ab = cpool.tile([HC, 8], BF16, tag=f"asdb{l}")
                nc.sync.dma_start(ab[:], asdb_in[l].ap())
                bt = cpool.tile([1, HC], F32, tag=f"bias{l}")
                nc.sync.dma_start(bt[:], bias_in[l].ap())
                wts[l] = (wt, ab, bt)
            ones1 = cpool.tile([1, 128], F32)
            nc.gpsimd.memset(ones1[:], 1.0)
            btf = {}
            for l in (1, 2, 3):
                bp = psC.tile([128, HC], F32, tag="ps2", name="bp")
                nc.tensor.matmul(bp[:], lhsT=ones1[:], rhs=wts[l][2][:],
                                 start=True, stop=True)
                btx = cpool.tile([128, HC], F32, tag=f"btf{l}", name="btx")
                nc.vector.tensor_copy(btx[:], bp[:])
                btf[l] = btx
            wl = cpool.tile([HC, A], F32)
            nc.sync.dma_start(wl[:], wl_in.ap())
            blv = cpool.tile([A, 1], F32)
            nc.sync.dma_start(blv[:], blv_in.ap())

            # sc_e for all 3 layers, combined slot layout: [128, W8*8, 12]
            sce3 = np1.tile([128, W8 * 8, 12], BF16, tag="sce3", name="sce3")
            for g in range(W8):
                a8 = sb.tile([128, 128], BF16, tag="attr8")
                nc.sync.dma_start(a8[:], attr8_in.ap()[g])
                pse = psB.tile([128, 96], F32, tag="ps1")
                nc.tensor.matmul(pse[:], lhsT=a8[:], rhs=qblk[:], start=True,
                                 stop=True)
                nc.scalar.copy(
                    sce3[:, g * 8:(g + 1) * 8, :]
                    .rearrange("p w (l h) -> p w l h", l=3),
                    pse[:].rearrange("p (l w h) -> p w l h", l=3, h=4))

            def node_phase(l, h_of, combined, sc_sd):
                wt, ab, _ = wts[l]
                F = F_IN if l == 1 else HC
                for t in range(NT):
                    hT = psB.tile([F, 128], BF16, tag="ps1")
                    nc.tensor.transpose(hT[:], h_of(t), identb[:])
                    hTs = sb.tile([F, 128], BF16, tag="hTs")
                    nc.scalar.copy(hTs[:], hT[:])
                    xwT = psC.tile([128, 128], F32, tag="ps2")
                    nc.tensor.matmul(xwT[:], lhsT=wt[:], rhs=hTs[:],
                                     start=True, stop=True)
                    xwTs = sb.tile([128, 128], BF16, tag="xwTs")
                    nc.scalar.copy(xwTs[:], xwT[:])
                    scp = psD.tile([128, 8], F32, tag="ps3")
                    nc.tensor.matmul(scp[:], lhsT=xwTs[:], rhs=ab[:],
                                     start=True, stop=True)
                    nc.vector.tensor_copy(sc_sd[:, t, :], scp[:])
                    xwN = psE.tile([128, 128], BF16, tag="ps4")
                    nc.tensor.transpose(xwN[:], xwTs[:], identb[:])
                    nc.scalar.copy(combined[:, t, 0:128], xwN[:])
                    nc.vector.tensor_copy(
                        combined[:, t, 128:136].bitcast(BF16), scp[:, 0:4])
                nc.sync.dma_start(
                    blk.ap().bitcast(F8)[:, 0:ROWB]
                    .rearrange("(t j) e -> j t e", j=128),
                    combined[:],
                )
                nc.sync.dma_start(blk.ap()[6250:NLOC, :], pois_in.ap())
                nc.gpsimd.collective_compute(
                    "AllGather", mybir.AluOpType.bypass, replica_groups=rg,
                    ins=[blk.ap()], outs=[tblS.ap()],
                )

            comb0 = np2.tile([128, NT, ROWB], F8, tag="comb")
            scsd0 = np2.tile([128, NT, 8], F32, tag="scsd")
            combined = {0: comb0, 1: None}
            sc_sd = {0: scsd0, 1: None}

            node_phase(1, lambda t: xblk[:, t, :], combined[0], sc_sd[0])

            hbuf = np1.tile([128, NT, HC], BF16, tag="hbuf", name="hbuf")

            for l in (1, 2, 3):
                cur = combined[(l - 1) % 2]
                cur_sc = sc_sd[(l - 1) % 2]
                _, _, bt = wts[l]

                # self-loop exp terms for the whole layer, batched:
                # sl = exp(leaky(msce_l + sc_s + sc_d))
                slx = np1.tile([128, NT, 4], F32, tag="slx", name="slx")
                nc.vector.tensor_add(slx[:], msce[:, :, (l - 1) * 4:l * 4],
                                     cur_sc[:, :, 0:4])
                nc.vector.tensor_add(slx[:], slx[:], cur_sc[:, :, 4:8])
                slp = np1.tile([128, NT, 4], F32, tag="slp", name="slp")
                nc.vector.tensor_scalar(slp[:], slx[:], NEG_SLOPE, None,
                                        mybir.AluOpType.mult)
                nc.vector.tensor_tensor(slx[:], slx[:], slp[:],
                                        mybir.AluOpType.max)
                sle = np1.tile([128, NT, 4], F32, tag="sle", name="sle")
                nc.scalar.activation(sle[:], slx[:],
                                     mybir.ActivationFunctionType.Exp)

                dnm_all = np1.tile([128, NT, 4], F32, tag="dnm", name="dnm")
                rec_all = np1.tile([128, NT, 4], F32, tag="rec", name="rec")

                for ch in chunks:
                    tiles = ch["tiles"]
                    tc_n = len(tiles)
                    ra, rb = ch["RA"], ch["RB"]
                    t0 = tiles[0]
                    nA, nB = tc_n * ra, tc_n * rb
                    nS = nA + nB
                    cA, cB = ch["colA0"], ch["colB0"]
                    gt = gp.tile([128, GW, ROWB], F8, tag="g")
                    nc.gpsimd.dma_gather(
                        out_ap=gt[:, :nA, :], in_ap=winA,
                        idxs_ap=gidx[:, cA * 8:(cA + nA) * 8],
                        num_idxs=nA * 128, num_idxs_reg=nA * 128,
                        elem_size=ROWB, elem_step=STRIDE, single_packet=False)
                    nc.gpsimd.dma_gather(
                        out_ap=gt[:, nA:nS, :], in_ap=winB,
                        idxs_ap=gidx[:, cB * 8:(cB + nB) * 8],
                        num_idxs=nB * 128, num_idxs_reg=nB * 128,
                        elem_size=ROWB, elem_step=STRIDE, single_packet=False)

                    # alpha for the whole chunk: pa = gath_scs + sce + sc_d
                    pa = pap.tile([128, GW, 4], F32, tag="pa", name="pa")
                    nc.vector.tensor_add(
                        pa[:, :nS, :],
                        gt[:, :nS, 128:136].bitcast(BF16),
                        sce3[:, cA:cA + nS, (l - 1) * 4:l * 4])
                    nc.vector.tensor_add(
                        pa[:, 0:nA, :].rearrange("p (t r) h -> p t r h", r=ra),
                        pa[:, 0:nA, :].rearrange("p (t r) h -> p t r h", r=ra),
                        cur_sc[:, t0:t0 + tc_n, 4:8].unsqueeze(2)
                        .to_broadcast([128, tc_n, ra, 4]))
                    nc.vector.tensor_add(
                        pa[:, nA:nS, :].rearrange("p (t r) h -> p t r h", r=rb),
                        pa[:, nA:nS, :].rearrange("p (t r) h -> p t r h", r=rb),
                        cur_sc[:, t0:t0 + tc_n, 4:8].unsqueeze(2)
                        .to_broadcast([128, tc_n, rb, 4]))
                    # leaky relu in one fused op: pa = max(pa*neg, pa)
                    nc.vector.scalar_tensor_tensor(
                        pa[:, :nS, :], pa[:, :nS, :], NEG_SLOPE, pa[:, :nS, :],
                        mybir.AluOpType.mult, mybir.AluOpType.max)
                    # exp -> bf16
                    pe = pap.tile([128, GW, 4], BF16, tag="pe", name="pe")
                    nc.scalar.activation(pe[:, :nS, :], pa[:, :nS, :],
                                         mybir.ActivationFunctionType.Exp)
                    # denominators per tile
                    dA = sb.tile([128, TCMAX, 4], F32, tag="dA", name="dA")
                    nc.vector.tensor_reduce(
                        dA[:, :tc_n, :],
                        pe[:, 0:nA, :].rearrange("p (t r) h -> p t h r", r=ra),
                        axis=mybir.AxisListType.X, op=mybir.AluOpType.add)
                    dB = sb.tile([128, TCMAX, 4], F32, tag="dB", name="dB")
                    nc.vector.tensor_reduce(
                        dB[:, :tc_n, :],
                        pe[:, nA:nS, :].rearrange("p (t r) h -> p t h r", r=rb),
                        axis=mybir.AxisListType.X, op=mybir.AluOpType.add)
                    nc.vector.tensor_add(dnm_all[:, t0:t0 + tc_n, :],
                                         dA[:, :tc_n, :], dB[:, :tc_n, :])

                    # transposed exp for the fast-path multiply
                    peT = pap.tile([128, 4, GW], BF16, tag="peT", name="peT")
                    nc.scalar.copy(peT[:, :, 0:nS],
                                   pe[:, :nS, :].rearrange("p s h -> p h s"))

                    # messages + aggregation per tile
                    aggp = psA.tile([128, TCMAX, HC], F32, tag="agg",
                                    name="aggp")
                    for tl, t in enumerate(tiles):
                        R = ra + rb
                        tA0, tB0 = tl * ra, nA + tl * rb
                        if t % 7 < ACT_CAST_MOD7:
                            # fast path: Act casts fp8->bf16 transposed, DVE
                            # multiplies in 2x mode (all 2-byte, stride-1)
                            gtb = sclp.tile([128, HC, RMAX], BF16, tag="gtbT",
                                            name="gtb")
                            nc.scalar.copy(
                                gtb[:, :, 0:ra],
                                gt[:, tA0:tA0 + ra, 0:128]
                                .rearrange("p r c -> p c r"))
                            nc.scalar.copy(
                                gtb[:, :, ra:R],
                                gt[:, tB0:tB0 + rb, 0:128]
                                .rearrange("p r c -> p c r"))
                            scl = sclp.tile([128, HC, RMAX], BF16, tag="sclT",
                                            name="sclT")
                            nc.vector.tensor_tensor(
                                scl[:, :, 0:ra]
                                .rearrange("p (h c) r -> p h c r", h=4),
                                gtb[:, :, 0:ra]
                                .rearrange("p (h c) r -> p h c r", h=4),
                                peT[:, :, tA0:tA0 + ra].unsqueeze(2)
                                .to_broadcast([128, 4, C, ra]),
                                mybir.AluOpType.mult)
                            nc.vector.tensor_tensor(
                                scl[:, :, ra:R]
                                .rearrange("p (h c) r -> p h c r", h=4),
                                gtb[:, :, ra:R]
                                .rearrange("p (h c) r -> p h c r", h=4),
                                peT[:, :, tB0:tB0 + rb].unsqueeze(2)
                                .to_broadcast([128, 4, C, rb]),
                                mybir.AluOpType.mult)
                            for r in range(R):
                                nc.tensor.matmul(
                                    aggp[:, tl, :], lhsT=identb[:],
                                    rhs=scl[:, :, r],
                                    start=(r == 0), stop=(r == R - 1))
                        else:
                            scl = sclp.tile([128, RMAX, HC], BF16, tag="scl",
                                            name="scl")
                            nc.vector.tensor_tensor(
                                scl[:, 0:ra, :]
                                .rearrange("p r (h c) -> p r h c", h=4),
                                gt[:, tA0:tA0 + ra, 0:128]
                                .rearrange("p r (h c) -> p r h c", h=4),
                                pe[:, tA0:tA0 + ra, :].unsqueeze(3)
                                .to_broadcast([128, ra, 4, C]),
                                mybir.AluOpType.mult)
                            nc.vector.tensor_tensor(
                                scl[:, ra:R, :]
                                .rearrange("p r (h c) -> p r h c", h=4),
                                gt[:, tB0:tB0 + rb, 0:128]
                                .rearrange("p r (h c) -> p r h c", h=4),
                                pe[:, tB0:tB0 + rb, :]
                                .unsqueeze(3).to_broadcast([128, rb, 4, C]),
                                mybir.AluOpType.mult)
                            for r in range(R):
                                nc.tensor.matmul(
                                    aggp[:, tl, :], lhsT=identb[:],
                                    rhs=scl[:, r, :],
                                    start=(r == 0), stop=(r == R - 1))

                    # finalize chunk: h = relu((agg + sle*xw_local)/dnm + bias)
                    nc.vector.tensor_add(
                        dnm_all[:, t0:t0 + tc_n, :],
                        dnm_all[:, t0:t0 + tc_n, :], sle[:, t0:t0 + tc_n, :])
                    nc.vector.tensor_scalar(
                        dnm_all[:, t0:t0 + tc_n, :],
                        dnm_all[:, t0:t0 + tc_n, :],
                        1e-16, None, mybir.AluOpType.add)
                    nc.vector.reciprocal(rec_all[:, t0:t0 + tc_n, :],
                                         dnm_all[:, t0:t0 + tc_n, :])
                    lt = sb.tile([128, TCMAX, HC], F32, tag="lt", name="lt")
                    nc.vector.tensor_tensor(
                        lt[:, :tc_n, :].rearrange("p t (h c) -> p t h c", h=4),
                        cur[:, t0:t0 + tc_n, 0:128]
                        .rearrange("p t (h c) -> p t h c", h=4),
                        sle[:, t0:t0 + tc_n, :].unsqueeze(3)
                        .to_broadcast([128, tc_n, 4, C]),
                        mybir.AluOpType.mult)
                    nc.vector.tensor_add(lt[:, :tc_n, :], lt[:, :tc_n, :],
                                         aggp[:, :tc_n, :])
                    nc.vector.tensor_tensor(
                        lt[:, :tc_n, :].rearrange("p t (h c) -> p t h c", h=4),
                        lt[:, :tc_n, :].rearrange("p t (h c) -> p t h c", h=4),
                        rec_all[:, t0:t0 + tc_n, :].unsqueeze(3)
                        .to_broadcast([128, tc_n, 4, C]),
                        mybir.AluOpType.mult)
                    nc.vector.tensor_add(
                        lt[:, :tc_n, :], lt[:, :tc_n, :],
                        btf[l][:].unsqueeze(1).to_broadcast([128, tc_n, HC]))
                    nc.vector.tensor_scalar(
                        hbuf[:, t0:t0 + tc_n, :], lt[:, :tc_n, :],
                        0.0, None, mybir.AluOpType.max)

                if l < 3:
                    combN = np2.tile([128, NT, ROWB], F8, tag="comb")
                    scsdN = np2.tile([128, NT, 8], F32, tag="scsd")
                    combined[l % 2] = combN
                    sc_sd[l % 2] = scsdN
                    node_phase(l + 1, lambda t: hbuf[:, t, :], combN, scsdN)
                else:
                    pmat = np1.tile([128, NT, B], BF16, tag="pmat",
                                    name="pmat")
                    nc.sync.dma_start(pmat[:], pmat_in.ap())
                    pl = psB.tile([HC, B], F32, tag="ps1")
                    for t in range(NT):
                        nc.tensor.matmul(pl[:], lhsT=hbuf[:, t, :],
                                         rhs=pmat[:, t, :],
                                         start=(t == 0), stop=(t == NT - 1))
                    pls = sb.tile([HC, B], F32, tag="pls")
                    nc.vector.tensor_copy(pls[:], pl[:])
                    nc.sync.dma_start(pool_in.ap(), pls[:])
                    nc.gpsimd.collective_compute(
                        "AllReduce", mybir.AluOpType.add, replica_groups=rg,
                        ins=[pool_in.ap()], outs=[pool_sh.ap()])
                    plr = sb.tile([HC, B], F32, tag="plr")
                    nc.sync.dma_start(plr[:], pool_sh.ap())
                    zt = psC.tile([A, B], F32, tag="ps2")
                    nc.tensor.matmul(zt[:], lhsT=wl[:], rhs=plr[:],
                                     start=True, stop=True)
                    ot = sb.tile([A, B], F32, tag="ot")
                    nc.scalar.activation(
                        ot[:], zt[:], mybir.ActivationFunctionType.Tanh,
                        bias=blv[:])
                    nc.sync.dma_start(out_t.ap(), ot[:])
    nc.compile()
    return nc


# ================================================================== entry
_CACHE = {}


def _get_nc(layout):
    key = (layout["WTOT"],
           tuple((tuple(ch["tiles"]), ch["RA"], ch["RB"]) for ch in layout["chunks"]))
    if key not in _CACHE:
        _CACHE[key] = build(layout)
    return _CACHE[key]


def kernel(**inputs):
    in_maps, layout = _prep(inputs)
    nc = _get_nc(layout)
    from concourse import bass2jax
    results = bass2jax.run_bass_via_pjrt(nc, in_maps, n_cores=NCORE)
    return np.ascontiguousarray(np.asarray(results[0]["out"], np.float32).T)


# revision 14
# speedup vs baseline: 1.0004x; 1.0004x over previous
"""GAT policy network (3-layer GAT + global mean pool head) on 8 Trainium2
NeuronCores via Bass/Tile.

Sharding: nodes are dealt to the 8 cores (graph/data parallel); each core owns
6250 dst nodes (padded to 6272 = 49 tiles x 128) and all edges incident on
them by destination.  Small GAT weights are replicated.

v2 design:
  * Table rows are [128 x fp8 xw | 4 x bf16 sc_s] = 136B payload at 256B
    stride, AllGather-replicated per layer; per-edge rows fetched with
    dma_gather through two int16-index windows (A: rows [0,32768), B: rows
    [17408, 50176)).
  * Chunk-uniform slot grid: the 49 dst tiles are partitioned into chunks of
    <= TCMAX tiles; within a chunk every tile gets the same A-slot and B-slot
    counts, so alpha / leaky-relu / exp / denominators batch into a handful of
    wide DVE/Act instructions per chunk instead of per-tile ops.
  * Self-loop scores use sc_e_loop = mean_attr @ (We a_e), precomputed on the
    host per layer -> the whole self-loop chain is a few batched ops.
  * Message aggregation: per tile, scl = gathered_xw * exp(alpha) (DVE), then
    either identity-matmul accumulation on PE or a strided tensor_reduce on
    DVE (split tunable to balance engines).
"""

import sys
sys.path.insert(0, '/opt/trn_rl_repo')

import inspect
import textwrap

import numpy as np
import ml_dtypes

import concourse.bass as bass
import concourse.bacc as bacc
import concourse.tile as tile
import concourse.mybir as mybir

bf16 = ml_dtypes.bfloat16
fp8 = ml_dtypes.float8_e4m3
F32 = mybir.dt.float32
BF16 = mybir.dt.bfloat16
F8 = mybir.dt.float8e4
I16 = mybir.dt.int16

# problem dims
N, E, F_IN, ED = 50000, 800000, 64, 16
H, C = 4, 32
HC = H * C
B, A = 64, 8
NEG_SLOPE = 0.2
NCORE = 8
NLOC = 6272
NT = 49
NTOT = NCORE * NLOC          # 50176
STRIDE = 256                 # table row stride, bytes (f8 elems)
ROWB = 136                   # gathered payload bytes per row
WINB = 17408
WINA_MAX = 32767
POISON = -1.0e38
POISON_A = 6271              # abs row, inside window A (padding row core 0)
POISON_B = 3 * NLOC + 6271   # abs row 25087, inside window B (padding row core 3)
TCMAX = 4                    # max tiles per chunk
ACT_CAST_MOD7 = 5            # of every 7 tiles, this many use the Act-cast 2x path


def _patch_dma_gather():
    """Relax the elem_size_bytes % 256 assert (transpose-only restriction; the
    non-transpose HBM path takes arbitrary payload length, only the row stride
    must be a multiple of 256B)."""
    if getattr(bass.BassGpSimd.dma_gather, "_gat_patched", False):
        return
    src = textwrap.dedent(inspect.getsource(bass.BassGpSimd.dma_gather))
    needle = (
        "    assert (\n"
        "        elem_size_bytes > 0 and elem_size_bytes % 256 == 0\n"
        "    )  # transpose restriction\n"
    )
    assert needle in src, "dma_gather source changed; patch needs update"
    src = src.replace(
        needle,
        "    assert elem_size_bytes > 0\n"
        "    if transpose:\n"
        "        assert elem_size_bytes % 256 == 0\n",
    )
    ns = vars(bass).copy()
    exec(compile(src, "<patched dma_gather>", "exec"), ns)
    fn = ns["dma_gather"]
    fn._gat_patched = True
    bass.BassGpSimd.dma_gather = fn


# ===================================================================== prep
def _prep(inputs):
    x = np.asarray(inputs["x"], np.float32)
    edge_attr = np.asarray(inputs["edge_attr"], np.float32)
    edge_index = np.asarray(inputs["edge_index"]).astype(np.int64)
    batch = np.asarray(inputs["batch"]).astype(np.int64)
    src, dst = edge_index[0], edge_index[1]

    deg = np.bincount(dst, minlength=N)
    odeg = np.bincount(src, minlength=N)

    # node -> core; put high out-degree nodes into cores whose table blocks
    # fall in the shared window region (cores 3,4,2,5 cover rows ~12.5K-37.6K)
    order = np.argsort(-odeg, kind="stable")
    owner = np.empty(N, np.int64)
    for i, c in enumerate([3, 4, 2, 5, 1, 6, 0, 7]):
        owner[order[i * 6250:(i + 1) * 6250]] = c

    local = np.empty(N, np.int64)
    nodes_of = []
    for c in range(NCORE):
        mine = np.where(owner == c)[0]
        mine = mine[np.argsort(-deg[mine], kind="stable")]
        local[mine] = np.arange(6250)
        nodes_of.append(mine)
    pos = owner * NLOC + local

    # --- per-core, per-node edge lists with A-only / B-only / free counts ---
    ecore = owner[dst]
    eloc = local[dst]
    # per (core, local node): lists of edges
    edges_of = [None] * NCORE
    nAonly = np.zeros((NCORE, NLOC), np.int64)
    nBonly = np.zeros((NCORE, NLOC), np.int64)
    ndeg = np.zeros((NCORE, NLOC), np.int64)
    for c in range(NCORE):
        sel = np.where(ecore == c)[0]
        d_loc = eloc[sel]
        ord2 = np.argsort(d_loc, kind="stable")
        sel = sel[ord2]
        d_loc = d_loc[ord2]
        spos = pos[src[sel]]
        okA = spos <= WINA_MAX
        okB = spos >= WINB
        bounds = np.searchsorted(d_loc, np.arange(NLOC + 1))
        edges_of[c] = (sel, bounds, okA, okB)
        cntA = np.zeros(NLOC, np.int64)
        cntB = np.zeros(NLOC, np.int64)
        cntD = np.zeros(NLOC, np.int64)
        np.add.at(cntA, d_loc[okA & ~okB], 1)
        np.add.at(cntB, d_loc[okB & ~okA], 1)
        np.add.at(cntD, d_loc, 1)
        nAonly[c] = cntA
        nBonly[c] = cntB
        ndeg[c] = cntD

    # per-tile global lower bounds
    LA = nAonly.reshape(NCORE, NT, 128).max(axis=(0, 2))
    LB = nBonly.reshape(NCORE, NT, 128).max(axis=(0, 2))
    LD = ndeg.reshape(NCORE, NT, 128).max(axis=(0, 2))

    # --- chunk the 49 tiles (DP, chunk size <= TCMAX, minimize padded slots)
    INF = 1 << 60
    CHUNK_COST = 56          # fixed per-chunk cost in slot-column equivalents
    best = [INF] * (NT + 1)
    prev = [0] * (NT + 1)
    best[0] = 0
    for j in range(1, NT + 1):
        for i in range(max(0, j - TCMAX), j):
            la = int(LA[i:j].max())
            lb = int(LB[i:j].max())
            ld = int(LD[i:j].max())
            w = max(ld, la + lb)
            cost = best[i] + (j - i) * w + CHUNK_COST
            if cost < best[j]:
                best[j] = cost
                prev[j] = i
    cuts = []
    j = NT
    while j > 0:
        cuts.append((prev[j], j))
        j = prev[j]
    cuts.reverse()

    chunks = []           # (tiles list, RA, RB, colA0, colB0)
    w = 0
    for (i, j) in cuts:
        la = int(LA[i:j].max())
        lb = int(LB[i:j].max())
        ld = int(LD[i:j].max())
        tot = max(ld, la + lb, 1)
        ra = la + (tot - la - lb + 1) // 2
        ra = max(ra, 1)
        rb = max(tot - ra, 1)
        tiles = list(range(i, j))
        colA0 = w
        colB0 = w + len(tiles) * ra
        w = colB0 + len(tiles) * rb
        chunks.append(dict(tiles=tiles, RA=ra, RB=rb, colA0=colA0, colB0=colB0))
    WTOT = w
    W8 = (WTOT + 7) // 8

    layout = dict(chunks=chunks, WTOT=WTOT, W8=W8)

    gcnt = np.bincount(batch, minlength=B).astype(np.float32)
    cnt = np.maximum(deg, 1.0)
    attr_sum = np.zeros((N, ED), np.float32)
    np.add.at(attr_sum, dst, edge_attr)
    mean_attr = attr_sum / cnt[:, None]

    # per-layer Q = We_l reshaped @ a_e_l  -> [ED, H]
    Qs = []
    for li in (1, 2, 3):
        Wem = np.asarray(inputs[f"We{li}"], np.float32)
        a_e = np.asarray(inputs[f"ae{li}"], np.float32)
        Q = np.zeros((ED, H), np.float32)
        for h in range(H):
            Q[:, h] = Wem[:, h * C:(h + 1) * C] @ a_e[h]
        Qs.append(Q)

    in_maps = []
    for c in range(NCORE):
        sel, bounds, okA, okB = edges_of[c]
        gidx = np.zeros((128, WTOT), np.int64)
        attr_rect = np.zeros((128, W8 * 8, ED), np.float32)
        for ch in chunks:
            ra, rb = ch["RA"], ch["RB"]
            for tl, t in enumerate(ch["tiles"]):
                ca = ch["colA0"] + tl * ra
                cb = ch["colB0"] + tl * rb
                for j in range(128):
                    ln = t * 128 + j
                    lo, hi = bounds[ln], bounds[ln + 1]
                    ea, eb = [], []
                    if lo < hi:
                        free = []
                        for k in range(lo, hi):
                            if okA[k] and okB[k]:
                                free.append(sel[k])
                            elif okA[k]:
                                ea.append(sel[k])
                            else:
                                eb.append(sel[k])
                        for e in free:
                            if len(ea) < ra and (len(ea) - nAonly[c][ln]
                                                 <= len(eb) - nBonly[c][ln]
                                                 or len(eb) >= rb):
                                ea.append(e)
                            else:
                                eb.append(e)
                    assert len(ea) <= ra and len(eb) <= rb, (c, t, j, len(ea), len(eb), ra, rb)
                    for r in range(ra):
                        if r < len(ea):
                            gidx[j, ca + r] = pos[src[ea[r]]]
                            attr_rect[j, ca + r] = edge_attr[ea[r]]
                        else:
                            gidx[j, ca + r] = POISON_A
                    for r in range(rb):
                        if r < len(eb):
                            gidx[j, cb + r] = pos[src[eb[r]]] - WINB
                            attr_rect[j, cb + r] = edge_attr[eb[r]]
                        else:
                            gidx[j, cb + r] = POISON_B - WINB
        assert 0 <= gidx.min() and gidx.max() <= 32767
        gidx = gidx.astype(np.int16)

        # wrapped idx layout: position i=(col-c0)*128+j -> idx16[j%16, col*8+j//16]
        jj = np.arange(128)
        gidxw = np.zeros((16, WTOT * 8), np.int16)
        cols8 = (np.arange(WTOT)[None, :] * 8 + (jj // 16)[:, None])  # [128, WTOT]
        gidxw[(jj % 16)[:, None], cols8] = gidx
        gidxw = np.tile(gidxw, (8, 1))

        # attr8[g, wj*ED+cc, j] = attr_rect[j, 8g+wj, cc]
        a4 = attr_rect.reshape(128, W8, 8, ED)
        attr8 = np.ascontiguousarray(
            a4.transpose(1, 2, 3, 0).reshape(W8, 8 * ED, 128)).astype(bf16)

        xblk = np.zeros((128, NT, F_IN), np.float32)
        pmat = np.zeros((128, NT, B), np.float32)
        msce = np.zeros((128, NT, 12), np.float32)
        mine = nodes_of[c]
        for ln in range(6250):
            t, j = ln // 128, ln % 128
            n = mine[ln]
            xblk[j, t] = x[n]
            pmat[j, t, batch[n]] = 1.0 / max(gcnt[batch[n]], 1.0)
            for li in range(3):
                msce[j, t, li * 4:(li + 1) * 4] = mean_attr[n] @ Qs[li]

        in_maps.append({"gidx": gidxw, "attr8": attr8,
                        "xblk": xblk.astype(bf16),
                        "pmat": pmat.astype(bf16), "msce": msce})

    # weights (replicated)
    wts = {}
    q8s = []
    for li, (Wk, ask, adk, bk) in enumerate(
            [("W1", "as1", "ad1", "b1"),
             ("W2", "as2", "ad2", "b2"),
             ("W3", "as3", "ad3", "b3")]):
        Wm = np.asarray(inputs[Wk], np.float32)
        a_s = np.asarray(inputs[ask], np.float32)
        a_d = np.asarray(inputs[adk], np.float32)
        bv = np.asarray(inputs[bk], np.float32)
        wts[f"w{li+1}"] = Wm.astype(bf16)
        asdb = np.zeros((HC, 8), np.float32)
        for h in range(H):
            asdb[h * C:(h + 1) * C, h] = a_s[h]
            asdb[h * C:(h + 1) * C, 4 + h] = a_d[h]
        wts[f"asdb{li+1}"] = asdb.astype(bf16)
        q8 = np.zeros((128, 32), np.float32)
        for wj in range(8):
            q8[wj * ED:(wj + 1) * ED, wj * 4:(wj + 1) * 4] = Qs[li]
        q8s.append(q8)
        wts[f"bias{li+1}"] = bv.reshape(1, HC)
    wts["qblk"] = np.concatenate(q8s, axis=1).astype(bf16)
    wts["wl"] = np.asarray(inputs["Wl"], np.float32)
    wts["blv"] = np.asarray(inputs["bl"], np.float32).reshape(A, 1)
    wts["ident"] = np.eye(128, dtype=np.float32)
    # poison block: full 256B rows for padding rows 6250..6271:
    # xw bytes = 0, sc_s bytes (128..136) = bf16(-1e38) x4
    pois = np.zeros((NLOC - 6250, 256), np.uint8)
    pb = np.full((4,), POISON, bf16).view(np.uint8)
    pois[:, 128:136] = np.tile(pb, (NLOC - 6250, 1))
    wts["poisblk"] = pois.view(bf16)
    for m in in_maps:
        m.update(wts)
    return in_maps, layout


# ==================================================================== build
def build(layout):
    _patch_dma_gather()
    chunks = layout["chunks"]
    WTOT, W8 = layout["WTOT"], layout["W8"]
    GW = max(len(ch["tiles"]) * (ch["RA"] + ch["RB"]) for ch in chunks)
    RMAX = max(ch["RA"] + ch["RB"] for ch in chunks)

    nc = bacc.Bacc("TRN2", target_bir_lowering=False, debug=False,
                   num_devices=NCORE)

    gidx_in = nc.dram_tensor("gidx", [128, WTOT * 8], I16, kind="ExternalInput")
    attr8_in = nc.dram_tensor("attr8", [W8, 128, 128], BF16, kind="ExternalInput")
    xblk_in = nc.dram_tensor("xblk", [128, NT, F_IN], BF16, kind="ExternalInput")
    pmat_in = nc.dram_tensor("pmat", [128, NT, B], BF16, kind="ExternalInput")
    msce_in = nc.dram_tensor("msce", [128, NT, 12], F32, kind="ExternalInput")
    w_in = {1: nc.dram_tensor("w1", [F_IN, HC], BF16, kind="ExternalInput"),
            2: nc.dram_tensor("w2", [HC, HC], BF16, kind="ExternalInput"),
            3: nc.dram_tensor("w3", [HC, HC], BF16, kind="ExternalInput")}
    asdb_in = {l: nc.dram_tensor(f"asdb{l}", [HC, 8], BF16, kind="ExternalInput")
               for l in (1, 2, 3)}
    bias_in = {l: nc.dram_tensor(f"bias{l}", [1, HC], F32, kind="ExternalInput")
               for l in (1, 2, 3)}
    qblk_in = nc.dram_tensor("qblk", [128, 96], BF16, kind="ExternalInput")
    wl_in = nc.dram_tensor("wl", [HC, A], F32, kind="ExternalInput")
    blv_in = nc.dram_tensor("blv", [A, 1], F32, kind="ExternalInput")
    ident_in = nc.dram_tensor("ident", [128, 128], F32, kind="ExternalInput")
    pois_in = nc.dram_tensor("poisblk", [NLOC - 6250, 128], BF16, kind="ExternalInput")
    out_t = nc.dram_tensor("out", [A, B], F32, kind="ExternalOutput")

    blk = nc.dram_tensor("blk", [NLOC, STRIDE // 2], BF16)
    tblS = nc.dram_tensor("tblS", [NTOT, STRIDE // 2], BF16, addr_space="Shared")
    pool_in = nc.dram_tensor("pool_in", [HC, B], F32)
    pool_sh = nc.dram_tensor("pool_sh", [HC, B], F32, addr_space="Shared")

    tblap = tblS.ap().bitcast(F8)
    winA = tblap[:, :ROWB]
    winB = tblap[WINB:, :ROWB]
    rg = [list(range(NCORE))]

    with tile.TileContext(nc) as tc:
        with (
            tc.tile_pool(name="const", bufs=1) as cpool,
            tc.tile_pool(name="sb", bufs=3) as sb,
            tc.tile_pool(name="sclp", bufs=2) as sclp,
            tc.tile_pool(name="gp", bufs=2) as gp,
            tc.tile_pool(name="pap", bufs=2) as pap,
            tc.tile_pool(name="np2", bufs=2) as np2,
            tc.tile_pool(name="np1", bufs=1) as np1,
            tc.tile_pool(name="psA", bufs=2, space="PSUM") as psA,
            tc.tile_pool(name="psB", bufs=2, space="PSUM") as psB,
            tc.tile_pool(name="psC", bufs=2, space="PSUM") as psC,
            tc.tile_pool(name="psD", bufs=1, space="PSUM") as psD,
            tc.tile_pool(name="psE", bufs=1, space="PSUM") as psE,
        ):
            identf = cpool.tile([128, 128], F32)
            nc.sync.dma_start(identf[:], ident_in.ap())
            identb = cpool.tile([128, 128], BF16)
            nc.vector.tensor_copy(identb[:], identf[:])
            gidx = cpool.tile([128, WTOT * 8], I16)
            nc.sync.dma_start(gidx[:], gidx_in.ap())
            qblk = cpool.tile([128, 96], BF16)
            nc.sync.dma_start(qblk[:], qblk_in.ap())
            xblk = cpool.tile([128, NT, F_IN], BF16)
            nc.sync.dma_start(xblk[:], xblk_in.ap())
            msce = cpool.tile([128, NT, 12], F32)
            nc.sync.dma_start(msce[:], msce_in.ap())
            wts = {}
            for l in (1, 2, 3):
                wt = cpool.tile([F_IN if l == 1 else HC, HC], BF16, tag=f"w{l}")
                nc.sync.dma_start(wt[:], w_in[l].ap())
                ab = cpool.tile([HC, 8], BF16, tag=f"asdb{l}")
                nc.sync.dma_start(ab[:], asdb_in[l].ap())
                bt = cpool.tile([1, HC], F32, tag=f"bias{l}")
                nc.sync.dma_start(bt[:], bias_in[l].ap())
                wts[l] = (wt, ab, bt)
            ones1 = cpool.tile([1, 128], F32)
            nc.gpsimd.memset(ones1[:], 1.0)
            btf = {}
            for l in (1, 2, 3):
                bp = psC.tile([128, HC], F32, tag="ps2", name="bp")
                nc.tensor.matmul(bp[:], lhsT=ones1[:], rhs=wts[l][2][:],
                                 start=True, stop=True)
                btx = cpool.tile([128, HC], F32, tag=f"btf{l}", name="btx")
                nc.vector.tensor_copy(btx[:], bp[:])
                btf[l] = btx
            wl = cpool.tile([HC, A], F32)
            nc.sync.dma_start(wl[:], wl_in.ap())
            blv = cpool.tile([A, 1], F32)
            nc.sync.dma_start(blv[:], blv_in.ap())

            # sc_e for all 3 layers, combined slot layout: [128, W8*8, 12]
            sce3 = np1.tile([128, W8 * 8, 12], BF16, tag="sce3", name="sce3")
            for g0 in range(0, W8, 4):
                gn = min(4, W8 - g0)
                a8 = sb.tile([128, 4, 128], BF16, tag="attr8")
                nc.sync.dma_start(a8[:, :gn, :], attr8_in.ap()[g0:g0 + gn])
                for gi in range(gn):
                    g = g0 + gi
                    pse = psB.tile([128, 96], F32, tag="ps1")
                    nc.tensor.matmul(pse[:], lhsT=a8[:, gi, :], rhs=qblk[:],
                                     start=True, stop=True)
                    eng = nc.vector if g % 2 == 0 else nc.scalar
                    cp = (eng.tensor_copy if g % 2 == 0 else eng.copy)
                    cp(sce3[:, g * 8:(g + 1) * 8, :]
                       .rearrange("p w (l h) -> p w l h", l=3),
                       pse[:].rearrange("p (l w h) -> p w l h", l=3, h=4))

            def node_phase(l, h_of, combined, sc_sd):
                wt, ab, _ = wts[l]
                F = F_IN if l == 1 else HC
                for t in range(NT):
                    hT = psB.tile([F, 128], BF16, tag="ps1")
                    nc.tensor.transpose(hT[:], h_of(t), identb[:])
                    hTs = sb.tile([F, 128], BF16, tag="hTs")
                    nc.scalar.copy(hTs[:], hT[:])
                    xwT = psC.tile([128, 128], F32, tag="ps2")
                    nc.tensor.matmul(xwT[:], lhsT=wt[:], rhs=hTs[:],
                                     start=True, stop=True)
                    xwTs = sb.tile([128, 128], BF16, tag="xwTs")
                    nc.scalar.copy(xwTs[:], xwT[:])
                    scp = psD.tile([128, 8], F32, tag="ps3")
                    nc.tensor.matmul(scp[:], lhsT=xwTs[:], rhs=ab[:],
                                     start=True, stop=True)
                    nc.vector.tensor_copy(sc_sd[:, t, :], scp[:])
                    xwN = psE.tile([128, 128], BF16, tag="ps4")
                    nc.tensor.transpose(xwN[:], xwTs[:], identb[:])
                    nc.scalar.copy(combined[:, t, 0:128], xwN[:])
                    nc.vector.tensor_copy(
                        combined[:, t, 128:136].bitcast(BF16), scp[:, 0:4])
                nc.sync.dma_start(
                    blk.ap().bitcast(F8)[:, 0:ROWB]
                    .rearrange("(t j) e -> j t e", j=128),
                    combined[:],
                )
                nc.sync.dma_start(blk.ap()[6250:NLOC, :], pois_in.ap())
                nc.gpsimd.collective_compute(
                    "AllGather", mybir.AluOpType.bypass, replica_groups=rg,
                    ins=[blk.ap()], outs=[tblS.ap()],
                )

            comb0 = np2.tile([128, NT, ROWB], F8, tag="comb")
            scsd0 = np2.tile([128, NT, 8], F32, tag="scsd")
            combined = {0: comb0, 1: None}
            sc_sd = {0: scsd0, 1: None}

            node_phase(1, lambda t: xblk[:, t, :], combined[0], sc_sd[0])

            hbuf = np1.tile([128, NT, HC], BF16, tag="hbuf", name="hbuf")

            for l in (1, 2, 3):
                cur = combined[(l - 1) % 2]
                cur_sc = sc_sd[(l - 1) % 2]
                _, _, bt = wts[l]

                # self-loop exp terms for the whole layer, batched:
                # sl = exp(leaky(msce_l + sc_s + sc_d))
                slx = np1.tile([128, NT, 4], F32, tag="slx", name="slx")
                nc.vector.tensor_add(slx[:], msce[:, :, (l - 1) * 4:l * 4],
                                     cur_sc[:, :, 0:4])
                nc.vector.tensor_add(slx[:], slx[:], cur_sc[:, :, 4:8])
                slp = np1.tile([128, NT, 4], F32, tag="slp", name="slp")
                nc.vector.tensor_scalar(slp[:], slx[:], NEG_SLOPE, None,
                                        mybir.AluOpType.mult)
                nc.vector.tensor_tensor(slx[:], slx[:], slp[:],
                                        mybir.AluOpType.max)
                sle = np1.tile([128, NT, 4], F32, tag="sle", name="sle")
                nc.scalar.activation(sle[:], slx[:],
                                     mybir.ActivationFunctionType.Exp)

                dnm_all = np1.tile([128, NT, 4], F32, tag="dnm", name="dnm")
                rec_all = np1.tile([128, NT, 4], F32, tag="rec", name="rec")

                for ch in chunks:
                    tiles = ch["tiles"]
                    tc_n = len(tiles)
                    ra, rb = ch["RA"], ch["RB"]
                    t0 = tiles[0]
                    nA, nB = tc_n * ra, tc_n * rb
                    nS = nA + nB
                    cA, cB = ch["colA0"], ch["colB0"]
                    gt = gp.tile([128, GW, ROWB], F8, tag="g")
                    nc.gpsimd.dma_gather(
                        out_ap=gt[:, :nA, :], in_ap=winA,
                        idxs_ap=gidx[:, cA * 8:(cA + nA) * 8],
                        num_idxs=nA * 128, num_idxs_reg=nA * 128,
                        elem_size=ROWB, elem_step=STRIDE, single_packet=False)
                    nc.gpsimd.dma_gather(
                        out_ap=gt[:, nA:nS, :], in_ap=winB,
                        idxs_ap=gidx[:, cB * 8:(cB + nB) * 8],
                        num_idxs=nB * 128, num_idxs_reg=nB * 128,
                        elem_size=ROWB, elem_step=STRIDE, single_packet=False)

                    # alpha for the whole chunk: pa = gath_scs + sce + sc_d
                    pa = pap.tile([128, GW, 4], F32, tag="pa", name="pa")
                    nc.vector.tensor_add(
                        pa[:, :nS, :],
                        gt[:, :nS, 128:136].bitcast(BF16),
                        sce3[:, cA:cA + nS, (l - 1) * 4:l * 4])
                    nc.vector.tensor_add(
                        pa[:, 0:nA, :].rearrange("p (t r) h -> p t r h", r=ra),
                        pa[:, 0:nA, :].rearrange("p (t r) h -> p t r h", r=ra),
                        cur_sc[:, t0:t0 + tc_n, 4:8].unsqueeze(2)
                        .to_broadcast([128, tc_n, ra, 4]))
                    nc.vector.tensor_add(
                        pa[:, nA:nS, :].rearrange("p (t r) h -> p t r h", r=rb),
                        pa[:, nA:nS, :].rearrange("p (t r) h -> p t r h", r=rb),
                        cur_sc[:, t0:t0 + tc_n, 4:8].unsqueeze(2)
                        .to_broadcast([128, tc_n, rb, 4]))
                    # leaky relu in one fused op: pa = max(pa*neg, pa)
                    nc.vector.scalar_tensor_tensor(
                        pa[:, :nS, :], pa[:, :nS, :], NEG_SLOPE, pa[:, :nS, :],
                        mybir.AluOpType.mult, mybir.AluOpType.max)
                    # exp -> bf16
                    pe = pap.tile([128, GW, 4], BF16, tag="pe", name="pe")
                    nc.scalar.activation(pe[:, :nS, :], pa[:, :nS, :],
                                         mybir.ActivationFunctionType.Exp)
                    # denominators per tile
                    dA = sb.tile([128, TCMAX, 4], F32, tag="dA", name="dA")
                    nc.vector.tensor_reduce(
                        dA[:, :tc_n, :],
                        pe[:, 0:nA, :].rearrange("p (t r) h -> p t h r", r=ra),
                        axis=mybir.AxisListType.X, op=mybir.AluOpType.add)
                    dB = sb.tile([128, TCMAX, 4], F32, tag="dB", name="dB")
                    nc.vector.tensor_reduce(
                        dB[:, :tc_n, :],
                        pe[:, nA:nS, :].rearrange("p (t r) h -> p t h r", r=rb),
                        axis=mybir.AxisListType.X, op=mybir.AluOpType.add)
                    nc.vector.tensor_add(dnm_all[:, t0:t0 + tc_n, :],
                                         dA[:, :tc_n, :], dB[:, :tc_n, :])

                    # transposed exp for the fast-path multiply
                    peT = pap.tile([128, 4, GW], BF16, tag="peT", name="peT")
                    nc.scalar.copy(peT[:, :, 0:nS],
                                   pe[:, :nS, :].rearrange("p s h -> p h s"))

                    # messages + aggregation per tile
                    aggp = psA.tile([128, TCMAX, HC], F32, tag="agg",
                                    name="aggp")
                    for tl, t in enumerate(tiles):
                        R = ra + rb
                        tA0, tB0 = tl * ra, nA + tl * rb
                        if t % 7 < ACT_CAST_MOD7:
                            # fast path: Act casts fp8->bf16 transposed, DVE
                            # multiplies in 2x mode (all 2-byte, stride-1)
                            gtb = sclp.tile([128, HC, RMAX], BF16, tag="gtbT",
                                            name="gtb")
                            nc.scalar.copy(
                                gtb[:, :, 0:ra],
                                gt[:, tA0:tA0 + ra, 0:128]
                                .rearrange("p r c -> p c r"))
                            nc.scalar.copy(
                                gtb[:, :, ra:R],
                                gt[:, tB0:tB0 + rb, 0:128]
                                .rearrange("p r c -> p c r"))
                            scl = sclp.tile([128, HC, RMAX], BF16, tag="sclT",
                                            name="sclT")
                            nc.vector.tensor_tensor(
                                scl[:, :, 0:ra]
                                .rearrange("p (h c) r -> p h c r", h=4),
                                gtb[:, :, 0:ra]
                                .rearrange("p (h c) r -> p h c r", h=4),
                                peT[:, :, tA0:tA0 + ra].unsqueeze(2)
                                .to_broadcast([128, 4, C, ra]),
                                mybir.AluOpType.mult)
                            nc.vector.tensor_tensor(
                                scl[:, :, ra:R]
                                .rearrange("p (h c) r -> p h c r", h=4),
                                gtb[:, :, ra:R]
                                .rearrange("p (h c) r -> p h c r", h=4),
                                peT[:, :, tB0:tB0 + rb].unsqueeze(2)
                                .to_broadcast([128, 4, C, rb]),
                                mybir.AluOpType.mult)
                            for r in range(R):
                                nc.tensor.matmul(
                                    aggp[:, tl, :], lhsT=identb[:],
                                    rhs=scl[:, :, r],
                                    start=(r == 0), stop=(r == R - 1))
                        else:
                            scl = sclp.tile([128, RMAX, HC], BF16, tag="scl",
                                            name="scl")
                            nc.vector.tensor_tensor(
                                scl[:, 0:ra, :]
                                .rearrange("p r (h c) -> p r h c", h=4),
                                gt[:, tA0:tA0 + ra, 0:128]
                                .rearrange("p r (h c) -> p r h c", h=4),
                                pe[:, tA0:tA0 + ra, :].unsqueeze(3)
                                .to_broadcast([128, ra, 4, C]),
                                mybir.AluOpType.mult)
                            nc.vector.tensor_tensor(
                                scl[:, ra:R, :]
                                .rearrange("p r (h c) -> p r h c", h=4),
                                gt[:, tB0:tB0 + rb, 0:128]
                                .rearrange("p r (h c) -> p r h c", h=4),
                                pe[:, tB0:tB0 + rb, :]
                                .unsqueeze(3).to_broadcast([128, rb, 4, C]),
                                mybir.AluOpType.mult)
                            for r in range(R):
                                nc.tensor.matmul(
                                    aggp[:, tl, :], lhsT=identb[:],
                                    rhs=scl[:, r, :],
                                    start=(r == 0), stop=(r == R - 1))

                    # finalize chunk: h = relu((agg + sle*xw_local)/dnm + bias)
                    nc.vector.tensor_add(
                        dnm_all[:, t0:t0 + tc_n, :],
                        dnm_all[:, t0:t0 + tc_n, :], sle[:, t0:t0 + tc_n, :])
                    nc.vector.tensor_scalar(
                        dnm_all[:, t0:t0 + tc_n, :],
                        dnm_all[:, t0:t0 + tc_n, :],
                        1e-16, None, mybir.AluOpType.add)
                    nc.vector.reciprocal(rec_all[:, t0:t0 + tc_n, :],
                                         dnm_all[:, t0:t0 + tc_n, :])
                    lt = sb.tile([128, TCMAX, HC], F32, tag="lt", name="lt")
                    nc.vector.tensor_tensor(
                        lt[:, :tc_n, :].rearrange("p t (h c) -> p t h c", h=4),
                        cur[:, t0:t0 + tc_n, 0:128]
                        .rearrange("p t (h c) -> p t h c", h=4),
                        sle[:, t0:t0 + tc_n, :].unsqueeze(3)
                        .to_broadcast([128, tc_n, 4, C]),
                        mybir.AluOpType.mult)
                    nc.vector.tensor_add(lt[:, :tc_n, :], lt[:, :tc_n, :],
                                         aggp[:, :tc_n, :])
                    nc.vector.tensor_tensor(
                        lt[:, :tc_n, :].rearrange("p t (h c) -> p t h c", h=4),
                        lt[:, :tc_n, :].rearrange("p t (h c) -> p t h c", h=4),
                        rec_all[:, t0:t0 + tc_n, :].unsqueeze(3)
                        .to_broadcast([128, tc_n, 4, C]),
                        mybir.AluOpType.mult)
                    nc.vector.tensor_add(
                        lt[:, :tc_n, :], lt[:, :tc_n, :],
                        btf[l][:].unsqueeze(1).to_broadcast([128, tc_n, HC]))
                    nc.vector.tensor_scalar(
                        hbuf[:, t0:t0 + tc_n, :], lt[:, :tc_n, :],
                        0.0, None, mybir.AluOpType.max)

                if l < 3:
                    combN = np2.tile([128, NT, ROWB], F8, tag="comb")
                    scsdN = np2.tile([128, NT, 8], F32, tag="scsd")
                    combined[l % 2] = combN
                    sc_sd[l % 2] = scsdN
                    node_phase(l + 1, lambda t: hbuf[:, t, :], combN, scsdN)
                else:
                    pmat = np1.tile([128, NT, B], BF16, tag="pmat",
                                    name="pmat")
                    nc.sync.dma_start(pmat[:], pmat_in.ap())
                    pl = psB.tile([HC, B], F32, tag="ps1")
                    for t in range(NT):
                        nc.tensor.matmul(pl[:], lhsT=hbuf[:, t, :],
                                         rhs=pmat[:, t, :],
                                         start=(t == 0), stop=(t == NT - 1))
                    pls = sb.tile([HC, B], F32, tag="pls")
                    nc.vector.tensor_copy(pls[:], pl[:])
                    nc.sync.dma_start(pool_in.ap(), pls[:])
                    nc.gpsimd.collective_compute(
                        "AllReduce", mybir.AluOpType.add, replica_groups=rg,
                        ins=[pool_in.ap()], outs=[pool_sh.ap()])
                    plr = sb.tile([HC, B], F32, tag="plr")
                    nc.sync.dma_start(plr[:], pool_sh.ap())
                    zt = psC.tile([A, B], F32, tag="ps2")
                    nc.tensor.matmul(zt[:], lhsT=wl[:], rhs=plr[:],
                                     start=True, stop=True)
                    ot = sb.tile([A, B], F32, tag="ot")
                    nc.scalar.activation(
                        ot[:], zt[:], mybir.ActivationFunctionType.Tanh,
                        bias=blv[:])
                    nc.sync.dma_start(out_t.ap(), ot[:])
    nc.compile()
    return nc


# ================================================================== entry
_CACHE = {}


def _get_nc(layout):
    key = (layout["WTOT"],
           tuple((tuple(ch["tiles"]), ch["RA"], ch["RB"]) for ch in layout["chunks"]))
    if key not in _CACHE:
        _CACHE[key] = build(layout)
    return _CACHE[key]


def kernel(**inputs):
    in_maps, layout = _prep(inputs)
    nc = _get_nc(layout)
    from concourse import bass2jax
    results = bass2jax.run_bass_via_pjrt(nc, in_maps, n_cores=NCORE)
    return np.ascontiguousarray(np.asarray(results[0]["out"], np.float32).T)


# revision 15
# speedup vs baseline: 1.1593x; 1.1589x over previous
"""GAT policy network (3-layer GAT + global mean pool head) on 8 Trainium2
NeuronCores via Bass/Tile.

Sharding: nodes are dealt to the 8 cores (graph/data parallel); each core owns
6250 dst nodes (padded to 6272 = 49 tiles x 128) and all edges incident on
them by destination.  Small GAT weights are replicated.

v2 design:
  * Table rows are [128 x fp8 xw | 4 x bf16 sc_s] = 136B payload at 256B
    stride, AllGather-replicated per layer; per-edge rows fetched with
    dma_gather through two int16-index windows (A: rows [0,32768), B: rows
    [17408, 50176)).
  * Chunk-uniform slot grid: the 49 dst tiles are partitioned into chunks of
    <= TCMAX tiles; within a chunk every tile gets the same A-slot and B-slot
    counts, so alpha / leaky-relu / exp / denominators batch into a handful of
    wide DVE/Act instructions per chunk instead of per-tile ops.
  * Self-loop scores use sc_e_loop = mean_attr @ (We a_e), precomputed on the
    host per layer -> the whole self-loop chain is a few batched ops.
  * Message aggregation: per tile, scl = gathered_xw * exp(alpha) (DVE), then
    either identity-matmul accumulation on PE or a strided tensor_reduce on
    DVE (split tunable to balance engines).
"""

import sys
sys.path.insert(0, '/opt/trn_rl_repo')

import inspect
import textwrap

import numpy as np
import ml_dtypes

import concourse.bass as bass
import concourse.bacc as bacc
import concourse.tile as tile
import concourse.mybir as mybir

bf16 = ml_dtypes.bfloat16
fp8 = ml_dtypes.float8_e4m3
F32 = mybir.dt.float32
BF16 = mybir.dt.bfloat16
F8 = mybir.dt.float8e4
I16 = mybir.dt.int16

# problem dims
N, E, F_IN, ED = 50000, 800000, 64, 16
H, C = 4, 32
HC = H * C
B, A = 64, 8
NEG_SLOPE = 0.2
NCORE = 8
NLOC = 6272
NT = 49
NTOT = NCORE * NLOC          # 50176
STRIDE = 256                 # table row stride, bytes (f8 elems)
ROWB = 136                   # gathered payload bytes per row
WINB = 17408
WINA_MAX = 32767
POISON = -1.0e38
POISON_A = 6271              # abs row, inside window A (padding row core 0)
POISON_B = 3 * NLOC + 6271   # abs row 25087, inside window B (padding row core 3)
TCMAX = 4                    # max tiles per chunk
ACT_CAST_MOD7 = 5            # of every 7 tiles, this many use the Act-cast 2x path


def _patch_dma_gather():
    """Relax the elem_size_bytes % 256 assert (transpose-only restriction; the
    non-transpose HBM path takes arbitrary payload length, only the row stride
    must be a multiple of 256B)."""
    if getattr(bass.BassGpSimd.dma_gather, "_gat_patched", False):
        return
    src = textwrap.dedent(inspect.getsource(bass.BassGpSimd.dma_gather))
    needle = (
        "    assert (\n"
        "        elem_size_bytes > 0 and elem_size_bytes % 256 == 0\n"
        "    )  # transpose restriction\n"
    )
    assert needle in src, "dma_gather source changed; patch needs update"
    src = src.replace(
        needle,
        "    assert elem_size_bytes > 0\n"
        "    if transpose:\n"
        "        assert elem_size_bytes % 256 == 0\n",
    )
    ns = vars(bass).copy()
    exec(compile(src, "<patched dma_gather>", "exec"), ns)
    fn = ns["dma_gather"]
    fn._gat_patched = True
    bass.BassGpSimd.dma_gather = fn


# ===================================================================== prep
def _prep(inputs):
    x = np.asarray(inputs["x"], np.float32)
    edge_attr = np.asarray(inputs["edge_attr"], np.float32)
    edge_index = np.asarray(inputs["edge_index"]).astype(np.int64)
    batch = np.asarray(inputs["batch"]).astype(np.int64)
    src, dst = edge_index[0], edge_index[1]

    deg = np.bincount(dst, minlength=N)
    odeg = np.bincount(src, minlength=N)

    # node -> core; put high out-degree nodes into cores whose table blocks
    # fall in the shared window region (cores 3,4,2,5 cover rows ~12.5K-37.6K)
    order = np.argsort(-odeg, kind="stable")
    owner = np.empty(N, np.int64)
    for i, c in enumerate([3, 4, 2, 5, 1, 6, 0, 7]):
        owner[order[i * 6250:(i + 1) * 6250]] = c

    local = np.empty(N, np.int64)
    nodes_of = []
    for c in range(NCORE):
        mine = np.where(owner == c)[0]
        mine = mine[np.argsort(-deg[mine], kind="stable")]
        local[mine] = np.arange(6250)
        nodes_of.append(mine)
    pos = owner * NLOC + local

    # --- per-core, per-node edge lists with A-only / B-only / free counts ---
    ecore = owner[dst]
    eloc = local[dst]
    # per (core, local node): lists of edges
    edges_of = [None] * NCORE
    nAonly = np.zeros((NCORE, NLOC), np.int64)
    nBonly = np.zeros((NCORE, NLOC), np.int64)
    ndeg = np.zeros((NCORE, NLOC), np.int64)
    for c in range(NCORE):
        sel = np.where(ecore == c)[0]
        d_loc = eloc[sel]
        ord2 = np.argsort(d_loc, kind="stable")
        sel = sel[ord2]
        d_loc = d_loc[ord2]
        spos = pos[src[sel]]
        okA = spos <= WINA_MAX
        okB = spos >= WINB
        bounds = np.searchsorted(d_loc, np.arange(NLOC + 1))
        edges_of[c] = (sel, bounds, okA, okB)
        cntA = np.zeros(NLOC, np.int64)
        cntB = np.zeros(NLOC, np.int64)
        cntD = np.zeros(NLOC, np.int64)
        np.add.at(cntA, d_loc[okA & ~okB], 1)
        np.add.at(cntB, d_loc[okB & ~okA], 1)
        np.add.at(cntD, d_loc, 1)
        nAonly[c] = cntA
        nBonly[c] = cntB
        ndeg[c] = cntD

    # per-tile global lower bounds
    LA = nAonly.reshape(NCORE, NT, 128).max(axis=(0, 2))
    LB = nBonly.reshape(NCORE, NT, 128).max(axis=(0, 2))
    LD = ndeg.reshape(NCORE, NT, 128).max(axis=(0, 2))

    # --- chunk the 49 tiles (DP, chunk size <= TCMAX, minimize padded slots)
    INF = 1 << 60
    CHUNK_COST = 56          # fixed per-chunk cost in slot-column equivalents
    best = [INF] * (NT + 1)
    prev = [0] * (NT + 1)
    best[0] = 0
    for j in range(1, NT + 1):
        for i in range(max(0, j - TCMAX), j):
            la = int(LA[i:j].max())
            lb = int(LB[i:j].max())
            ld = int(LD[i:j].max())
            w = max(ld, la + lb)
            cost = best[i] + (j - i) * w + CHUNK_COST
            if cost < best[j]:
                best[j] = cost
                prev[j] = i
    cuts = []
    j = NT
    while j > 0:
        cuts.append((prev[j], j))
        j = prev[j]
    cuts.reverse()

    chunks = []           # (tiles list, RA, RB, colA0, colB0)
    w = 0
    for (i, j) in cuts:
        la = int(LA[i:j].max())
        lb = int(LB[i:j].max())
        ld = int(LD[i:j].max())
        tot = max(ld, la + lb, 1)
        ra = la + (tot - la - lb + 1) // 2
        ra = max(ra, 1)
        rb = max(tot - ra, 1)
        tiles = list(range(i, j))
        colA0 = w
        colB0 = w + len(tiles) * ra
        w = colB0 + len(tiles) * rb
        chunks.append(dict(tiles=tiles, RA=ra, RB=rb, colA0=colA0, colB0=colB0))
    WTOT = w
    W8 = (WTOT + 7) // 8

    layout = dict(chunks=chunks, WTOT=WTOT, W8=W8)

    gcnt = np.bincount(batch, minlength=B).astype(np.float32)
    cnt = np.maximum(deg, 1.0)
    attr_sum = np.zeros((N, ED), np.float32)
    np.add.at(attr_sum, dst, edge_attr)
    mean_attr = attr_sum / cnt[:, None]

    # per-layer Q = We_l reshaped @ a_e_l  -> [ED, H]
    Qs = []
    for li in (1, 2, 3):
        Wem = np.asarray(inputs[f"We{li}"], np.float32)
        a_e = np.asarray(inputs[f"ae{li}"], np.float32)
        Q = np.zeros((ED, H), np.float32)
        for h in range(H):
            Q[:, h] = Wem[:, h * C:(h + 1) * C] @ a_e[h]
        Qs.append(Q)

    in_maps = []
    for c in range(NCORE):
        sel, bounds, okA, okB = edges_of[c]
        gidx = np.zeros((128, WTOT), np.int64)
        attr_rect = np.zeros((128, W8 * 8, ED), np.float32)
        for ch in chunks:
            ra, rb = ch["RA"], ch["RB"]
            for tl, t in enumerate(ch["tiles"]):
                ca = ch["colA0"] + tl * ra
                cb = ch["colB0"] + tl * rb
                for j in range(128):
                    ln = t * 128 + j
                    lo, hi = bounds[ln], bounds[ln + 1]
                    ea, eb = [], []
                    if lo < hi:
                        free = []
                        for k in range(lo, hi):
                            if okA[k] and okB[k]:
                                free.append(sel[k])
                            elif okA[k]:
                                ea.append(sel[k])
                            else:
                                eb.append(sel[k])
                        for e in free:
                            if len(ea) < ra and (len(ea) - nAonly[c][ln]
                                                 <= len(eb) - nBonly[c][ln]
                                                 or len(eb) >= rb):
                                ea.append(e)
                            else:
                                eb.append(e)
                    assert len(ea) <= ra and len(eb) <= rb, (c, t, j, len(ea), len(eb), ra, rb)
                    for r in range(ra):
                        if r < len(ea):
                            gidx[j, ca + r] = pos[src[ea[r]]]
                            attr_rect[j, ca + r] = edge_attr[ea[r]]
                        else:
                            gidx[j, ca + r] = POISON_A
                    for r in range(rb):
                        if r < len(eb):
                            gidx[j, cb + r] = pos[src[eb[r]]] - WINB
                            attr_rect[j, cb + r] = edge_attr[eb[r]]
                        else:
                            gidx[j, cb + r] = POISON_B - WINB
        assert 0 <= gidx.min() and gidx.max() <= 32767
        gidx = gidx.astype(np.int16)

        # wrapped idx layout: position i=(col-c0)*128+j -> idx16[j%16, col*8+j//16]
        jj = np.arange(128)
        gidxw = np.zeros((16, WTOT * 8), np.int16)
        cols8 = (np.arange(WTOT)[None, :] * 8 + (jj // 16)[:, None])  # [128, WTOT]
        gidxw[(jj % 16)[:, None], cols8] = gidx
        gidxw = np.tile(gidxw, (8, 1))

        # attr8[g, wj*ED+cc, j] = attr_rect[j, 8g+wj, cc]
        a4 = attr_rect.reshape(128, W8, 8, ED)
        attr8 = np.ascontiguousarray(
            a4.transpose(1, 2, 3, 0).reshape(W8, 8 * ED, 128)).astype(bf16)

        xblk = np.zeros((128, NT, F_IN), np.float32)
        pmat = np.zeros((128, NT, B), np.float32)
        msce = np.zeros((128, NT, 12), np.float32)
        mine = nodes_of[c]
        for ln in range(6250):
            t, j = ln // 128, ln % 128
            n = mine[ln]
            xblk[j, t] = x[n]
            pmat[j, t, batch[n]] = 1.0 / max(gcnt[batch[n]], 1.0)
            for li in range(3):
                msce[j, t, li * 4:(li + 1) * 4] = mean_attr[n] @ Qs[li]

        in_maps.append({"gidx": gidxw, "attr8": attr8,
                        "xblk": xblk.astype(bf16),
                        "pmat": pmat.astype(bf16), "msce": msce})

    # weights (replicated)
    wts = {}
    q8s = []
    for li, (Wk, ask, adk, bk) in enumerate(
            [("W1", "as1", "ad1", "b1"),
             ("W2", "as2", "ad2", "b2"),
             ("W3", "as3", "ad3", "b3")]):
        Wm = np.asarray(inputs[Wk], np.float32)
        a_s = np.asarray(inputs[ask], np.float32)
        a_d = np.asarray(inputs[adk], np.float32)
        bv = np.asarray(inputs[bk], np.float32)
        wts[f"w{li+1}"] = Wm.astype(bf16)
        asdb = np.zeros((HC, 8), np.float32)
        for h in range(H):
            asdb[h * C:(h + 1) * C, h] = a_s[h]
            asdb[h * C:(h + 1) * C, 4 + h] = a_d[h]
        wts[f"asdb{li+1}"] = asdb.astype(bf16)
        q8 = np.zeros((128, 32), np.float32)
        for wj in range(8):
            q8[wj * ED:(wj + 1) * ED, wj * 4:(wj + 1) * 4] = Qs[li]
        q8s.append(q8)
        wts[f"bias{li+1}"] = bv.reshape(1, HC)
    wts["qblk"] = np.concatenate(q8s, axis=1).astype(bf16)
    wts["wl"] = np.asarray(inputs["Wl"], np.float32)
    wts["blv"] = np.asarray(inputs["bl"], np.float32).reshape(A, 1)
    wts["ident"] = np.eye(128, dtype=np.float32)
    # poison block: full 256B rows for padding rows 6250..6271:
    # xw bytes = 0, sc_s bytes (128..136) = bf16(-1e38) x4
    pois = np.zeros((NLOC - 6250, 256), np.uint8)
    pb = np.full((4,), POISON, bf16).view(np.uint8)
    pois[:, 128:136] = np.tile(pb, (NLOC - 6250, 1))
    wts["poisblk"] = pois.view(bf16)
    for m in in_maps:
        m.update(wts)
    return in_maps, layout


# ==================================================================== build
def build(layout):
    _patch_dma_gather()
    chunks = layout["chunks"]
    WTOT, W8 = layout["WTOT"], layout["W8"]
    GW = max(len(ch["tiles"]) * (ch["RA"] + ch["RB"]) for ch in chunks)
    RMAX = max(ch["RA"] + ch["RB"] for ch in chunks)

    nc = bacc.Bacc("TRN2", target_bir_lowering=False, debug=False,
                   num_devices=NCORE)

    gidx_in = nc.dram_tensor("gidx", [128, WTOT * 8], I16, kind="ExternalInput")
    attr8_in = nc.dram_tensor("attr8", [W8, 128, 128], BF16, kind="ExternalInput")
    xblk_in = nc.dram_tensor("xblk", [128, NT, F_IN], BF16, kind="ExternalInput")
    pmat_in = nc.dram_tensor("pmat", [128, NT, B], BF16, kind="ExternalInput")
    msce_in = nc.dram_tensor("msce", [128, NT, 12], F32, kind="ExternalInput")
    w_in = {1: nc.dram_tensor("w1", [F_IN, HC], BF16, kind="ExternalInput"),
            2: nc.dram_tensor("w2", [HC, HC], BF16, kind="ExternalInput"),
            3: nc.dram_tensor("w3", [HC, HC], BF16, kind="ExternalInput")}
    asdb_in = {l: nc.dram_tensor(f"asdb{l}", [HC, 8], BF16, kind="ExternalInput")
               for l in (1, 2, 3)}
    bias_in = {l: nc.dram_tensor(f"bias{l}", [1, HC], F32, kind="ExternalInput")
               for l in (1, 2, 3)}
    qblk_in = nc.dram_tensor("qblk", [128, 96], BF16, kind="ExternalInput")
    wl_in = nc.dram_tensor("wl", [HC, A], F32, kind="ExternalInput")
    blv_in = nc.dram_tensor("blv", [A, 1], F32, kind="ExternalInput")
    ident_in = nc.dram_tensor("ident", [128, 128], F32, kind="ExternalInput")
    pois_in = nc.dram_tensor("poisblk", [NLOC - 6250, 128], BF16, kind="ExternalInput")
    out_t = nc.dram_tensor("out", [A, B], F32, kind="ExternalOutput")

    blk = nc.dram_tensor("blk", [NLOC, STRIDE // 2], BF16)
    tblS = nc.dram_tensor("tblS", [NTOT, STRIDE // 2], BF16, addr_space="Shared")
    pool_in = nc.dram_tensor("pool_in", [HC, B], F32)
    pool_sh = nc.dram_tensor("pool_sh", [HC, B], F32, addr_space="Shared")

    tblap = tblS.ap().bitcast(F8)
    winA = tblap[:, :ROWB]
    winB = tblap[WINB:, :ROWB]
    rg = [list(range(NCORE))]

    with tile.TileContext(nc) as tc:
        with (
            tc.tile_pool(name="const", bufs=1) as cpool,
            tc.tile_pool(name="sb", bufs=3) as sb,
            tc.tile_pool(name="sclp", bufs=2) as sclp,
            tc.tile_pool(name="gp", bufs=2) as gp,
            tc.tile_pool(name="pap", bufs=2) as pap,
            tc.tile_pool(name="np2", bufs=2) as np2,
            tc.tile_pool(name="np1", bufs=1) as np1,
            tc.tile_pool(name="psA", bufs=2, space="PSUM") as psA,
            tc.tile_pool(name="psB", bufs=2, space="PSUM") as psB,
            tc.tile_pool(name="psC", bufs=2, space="PSUM") as psC,
            tc.tile_pool(name="psD", bufs=1, space="PSUM") as psD,
            tc.tile_pool(name="psE", bufs=1, space="PSUM") as psE,
        ):
            identf = cpool.tile([128, 128], F32)
            nc.sync.dma_start(identf[:], ident_in.ap())
            identb = cpool.tile([128, 128], BF16)
            nc.vector.tensor_copy(identb[:], identf[:])
            gidx = cpool.tile([128, WTOT * 8], I16)
            nc.sync.dma_start(gidx[:], gidx_in.ap())
            qblk = cpool.tile([128, 96], BF16)
            nc.sync.dma_start(qblk[:], qblk_in.ap())
            xblk = cpool.tile([128, NT, F_IN], BF16)
            nc.sync.dma_start(xblk[:], xblk_in.ap())
            msce = cpool.tile([128, NT, 12], F32)
            nc.sync.dma_start(msce[:], msce_in.ap())
            wts = {}
            for l in (1, 2, 3):
                wt = cpool.tile([F_IN if l == 1 else HC, HC], BF16, tag=f"w{l}")
                nc.sync.dma_start(wt[:], w_in[l].ap())
                ab = cpool.tile([HC, 8], BF16, tag=f"asdb{l}")
                nc.sync.dma_start(ab[:], asdb_in[l].ap())
                bt = cpool.tile([1, HC], F32, tag=f"bias{l}")
                nc.sync.dma_start(bt[:], bias_in[l].ap())
                wts[l] = (wt, ab, bt)
            ones1 = cpool.tile([1, 128], F32)
            nc.gpsimd.memset(ones1[:], 1.0)
            btf = {}
            for l in (1, 2, 3):
                bp = psC.tile([128, HC], F32, tag="ps2", name="bp")
                nc.tensor.matmul(bp[:], lhsT=ones1[:], rhs=wts[l][2][:],
                                 start=True, stop=True)
                btx = cpool.tile([128, HC], F32, tag=f"btf{l}", name="btx")
                nc.vector.tensor_copy(btx[:], bp[:])
                btf[l] = btx
            wl = cpool.tile([HC, A], F32)
            nc.sync.dma_start(wl[:], wl_in.ap())
            blv = cpool.tile([A, 1], F32)
            nc.sync.dma_start(blv[:], blv_in.ap())

            # sc_e for all 3 layers, combined slot layout: [128, W8*8, 12]
            sce3 = np1.tile([128, W8 * 8, 12], BF16, tag="sce3", name="sce3")
            for g in range(W8):
                a8 = sb.tile([128, 128], BF16, tag="attr8")
                nc.sync.dma_start(a8[:], attr8_in.ap()[g])
                pse = psB.tile([128, 96], F32, tag="ps1")
                nc.tensor.matmul(pse[:], lhsT=a8[:], rhs=qblk[:], start=True,
                                 stop=True)
                nc.scalar.copy(
                    sce3[:, g * 8:(g + 1) * 8, :]
                    .rearrange("p w (l h) -> p w l h", l=3),
                    pse[:].rearrange("p (l w h) -> p w l h", l=3, h=4))

            def node_phase(l, h_of, combined, sc_sd):
                wt, ab, _ = wts[l]
                F = F_IN if l == 1 else HC
                for t in range(NT):
                    hT = psB.tile([F, 128], BF16, tag="ps1")
                    nc.tensor.transpose(hT[:], h_of(t), identb[:])
                    hTs = sb.tile([F, 128], BF16, tag="hTs")
                    nc.scalar.copy(hTs[:], hT[:])
                    xwT = psC.tile([128, 128], F32, tag="ps2")
                    nc.tensor.matmul(xwT[:], lhsT=wt[:], rhs=hTs[:],
                                     start=True, stop=True)
                    xwTs = sb.tile([128, 128], BF16, tag="xwTs")
                    nc.scalar.copy(xwTs[:], xwT[:])
                    scp = psD.tile([128, 8], F32, tag="ps3")
                    nc.tensor.matmul(scp[:], lhsT=xwTs[:], rhs=ab[:],
                                     start=True, stop=True)
                    nc.vector.tensor_copy(sc_sd[:, t, :], scp[:])
                    xwN = psE.tile([128, 128], BF16, tag="ps4")
                    nc.tensor.transpose(xwN[:], xwTs[:], identb[:])
                    nc.scalar.copy(combined[:, t, 0:128], xwN[:])
                    nc.vector.tensor_copy(
                        combined[:, t, 128:136].bitcast(BF16), scp[:, 0:4])
                nc.sync.dma_start(
                    blk.ap().bitcast(F8)[:, 0:ROWB]
                    .rearrange("(t j) e -> j t e", j=128),
                    combined[:],
                )
                nc.sync.dma_start(blk.ap()[6250:NLOC, :], pois_in.ap())
                nc.gpsimd.collective_compute(
                    "AllGather", mybir.AluOpType.bypass, replica_groups=rg,
                    ins=[blk.ap()], outs=[tblS.ap()],
                )

            comb0 = np2.tile([128, NT, ROWB], F8, tag="comb")
            scsd0 = np2.tile([128, NT, 8], F32, tag="scsd")
            combined = {0: comb0, 1: None}
            sc_sd = {0: scsd0, 1: None}

            node_phase(1, lambda t: xblk[:, t, :], combined[0], sc_sd[0])

            hbuf = np1.tile([128, NT, HC], BF16, tag="hbuf", name="hbuf")

            for l in (1, 2, 3):
                cur = combined[(l - 1) % 2]
                cur_sc = sc_sd[(l - 1) % 2]
                _, _, bt = wts[l]

                # self-loop exp terms for the whole layer, batched:
                # sl = exp(leaky(msce_l + sc_s + sc_d))
                slx = np1.tile([128, NT, 4], F32, tag="slx", name="slx")
                nc.vector.tensor_add(slx[:], msce[:, :, (l - 1) * 4:l * 4],
                                     cur_sc[:, :, 0:4])
                nc.vector.tensor_add(slx[:], slx[:], cur_sc[:, :, 4:8])
                slp = np1.tile([128, NT, 4], F32, tag="slp", name="slp")
                nc.vector.tensor_scalar(slp[:], slx[:], NEG_SLOPE, None,
                                        mybir.AluOpType.mult)
                nc.vector.tensor_tensor(slx[:], slx[:], slp[:],
                                        mybir.AluOpType.max)
                sle = np1.tile([128, NT, 4], F32, tag="sle", name="sle")
                nc.scalar.activation(sle[:], slx[:],
                                     mybir.ActivationFunctionType.Exp)

                dnm_all = np1.tile([128, NT, 4], F32, tag="dnm", name="dnm")
                rec_all = np1.tile([128, NT, 4], F32, tag="rec", name="rec")

                for ch in chunks:
                    tiles = ch["tiles"]
                    tc_n = len(tiles)
                    ra, rb = ch["RA"], ch["RB"]
                    t0 = tiles[0]
                    nA, nB = tc_n * ra, tc_n * rb
                    nS = nA + nB
                    cA, cB = ch["colA0"], ch["colB0"]
                    gt = gp.tile([128, GW, ROWB], F8, tag="g")
                    nc.gpsimd.dma_gather(
                        out_ap=gt[:, :nA, :], in_ap=winA,
                        idxs_ap=gidx[:, cA * 8:(cA + nA) * 8],
                        num_idxs=nA * 128, num_idxs_reg=nA * 128,
                        elem_size=ROWB, elem_step=STRIDE, single_packet=False)
                    nc.gpsimd.dma_gather(
                        out_ap=gt[:, nA:nS, :], in_ap=winB,
                        idxs_ap=gidx[:, cB * 8:(cB + nB) * 8],
                        num_idxs=nB * 128, num_idxs_reg=nB * 128,
                        elem_size=ROWB, elem_step=STRIDE, single_packet=False)

                    # alpha for the whole chunk: pa = gath_scs + sce + sc_d
                    pa = pap.tile([128, GW, 4], F32, tag="pa", name="pa")
                    nc.vector.tensor_add(
                        pa[:, :nS, :],
                        gt[:, :nS, 128:136].bitcast(BF16),
                        sce3[:, cA:cA + nS, (l - 1) * 4:l * 4])
                    nc.vector.tensor_add(
                        pa[:, 0:nA, :].rearrange("p (t r) h -> p t r h", r=ra),
                        pa[:, 0:nA, :].rearrange("p (t r) h -> p t r h", r=ra),
                        cur_sc[:, t0:t0 + tc_n, 4:8].unsqueeze(2)
                        .to_broadcast([128, tc_n, ra, 4]))
                    nc.vector.tensor_add(
                        pa[:, nA:nS, :].rearrange("p (t r) h -> p t r h", r=rb),
                        pa[:, nA:nS, :].rearrange("p (t r) h -> p t r h", r=rb),
                        cur_sc[:, t0:t0 + tc_n, 4:8].unsqueeze(2)
                        .to_broadcast([128, tc_n, rb, 4]))
                    # leaky relu in one fused op: pa = max(pa*neg, pa)
                    nc.vector.scalar_tensor_tensor(
                        pa[:, :nS, :], pa[:, :nS, :], NEG_SLOPE, pa[:, :nS, :],
                        mybir.AluOpType.mult, mybir.AluOpType.max)
                    # exp -> bf16
                    pe = pap.tile([128, GW, 4], BF16, tag="pe", name="pe")
                    nc.scalar.activation(pe[:, :nS, :], pa[:, :nS, :],
                                         mybir.ActivationFunctionType.Exp)
                    # denominators per tile
                    dA = sb.tile([128, TCMAX, 4], F32, tag="dA", name="dA")
                    nc.vector.tensor_reduce(
                        dA[:, :tc_n, :],
                        pe[:, 0:nA, :].rearrange("p (t r) h -> p t h r", r=ra),
                        axis=mybir.AxisListType.X, op=mybir.AluOpType.add)
                    dB = sb.tile([128, TCMAX, 4], F32, tag="dB", name="dB")
                    nc.vector.tensor_reduce(
                        dB[:, :tc_n, :],
                        pe[:, nA:nS, :].rearrange("p (t r) h -> p t h r", r=rb),
                        axis=mybir.AxisListType.X, op=mybir.AluOpType.add)
                    nc.vector.tensor_add(dnm_all[:, t0:t0 + tc_n, :],
                                         dA[:, :tc_n, :], dB[:, :tc_n, :])

                    # transposed exp for the fast-path multiply
                    peT = pap.tile([128, 4, GW], BF16, tag="peT", name="peT")
                    nc.scalar.copy(peT[:, :, 0:nS],
                                   pe[:, :nS, :].rearrange("p s h -> p h s"))

                    # messages + aggregation per tile
                    aggp = psA.tile([128, TCMAX, HC], F32, tag="agg",
                                    name="aggp")
                    for tl, t in enumerate(tiles):
                        R = ra + rb
                        tA0, tB0 = tl * ra, nA + tl * rb
                        if t % 7 < ACT_CAST_MOD7:
                            # fast path: Act casts fp8->bf16 transposed, DVE
                            # multiplies in 2x mode (all 2-byte, stride-1)
                            gtb = sclp.tile([128, HC, RMAX], BF16, tag="gtbT",
                                            name="gtb")
                            nc.scalar.copy(
                                gtb[:, :, 0:ra],
                                gt[:, tA0:tA0 + ra, 0:128]
                                .rearrange("p r c -> p c r"))
                            nc.scalar.copy(
                                gtb[:, :, ra:R],
                                gt[:, tB0:tB0 + rb, 0:128]
                                .rearrange("p r c -> p c r"))
                            scl = sclp.tile([128, HC, RMAX], BF16, tag="sclT",
                                            name="sclT")
                            nc.vector.tensor_tensor(
                                scl[:, :, 0:ra]
                                .rearrange("p (h c) r -> p h c r", h=4),
                                gtb[:, :, 0:ra]
                                .rearrange("p (h c) r -> p h c r", h=4),
                                peT[:, :, tA0:tA0 + ra].unsqueeze(2)
                                .to_broadcast([128, 4, C, ra]),
                                mybir.AluOpType.mult)
                            nc.vector.tensor_tensor(
                                scl[:, :, ra:R]
                                .rearrange("p (h c) r -> p h c r", h=4),
                                gtb[:, :, ra:R]
                                .rearrange("p (h c) r -> p h c r", h=4),
                                peT[:, :, tB0:tB0 + rb].unsqueeze(2)
                                .to_broadcast([128, 4, C, rb]),
                                mybir.AluOpType.mult)
                            for r in range(R):
                                nc.tensor.matmul(
                                    aggp[:, tl, :], lhsT=identb[:],
                                    rhs=scl[:, :, r],
                                    start=(r == 0), stop=(r == R - 1))
                        else:
                            scl = sclp.tile([128, RMAX, HC], BF16, tag="scl",
                                            name="scl")
                            nc.vector.tensor_tensor(
                                scl[:, 0:ra, :]
                                .rearrange("p r (h c) -> p r h c", h=4),
                                gt[:, tA0:tA0 + ra, 0:128]
                                .rearrange("p r (h c) -> p r h c", h=4),
                                pe[:, tA0:tA0 + ra, :].unsqueeze(3)
                                .to_broadcast([128, ra, 4, C]),
                                mybir.AluOpType.mult)
                            nc.vector.tensor_tensor(
                                scl[:, ra:R, :]
                                .rearrange("p r (h c) -> p r h c", h=4),
                                gt[:, tB0:tB0 + rb, 0:128]
                                .rearrange("p r (h c) -> p r h c", h=4),
                                pe[:, tB0:tB0 + rb, :]
                                .unsqueeze(3).to_broadcast([128, rb, 4, C]),
                                mybir.AluOpType.mult)
                            for r in range(R):
                                nc.tensor.matmul(
                                    aggp[:, tl, :], lhsT=identb[:],
                                    rhs=scl[:, r, :],
                                    start=(r == 0), stop=(r == R - 1))

                    # finalize chunk: h = relu((agg + sle*xw_local)/dnm + bias)
                    nc.vector.tensor_add(
                        dnm_all[:, t0:t0 + tc_n, :],
                        dnm_all[:, t0:t0 + tc_n, :], sle[:, t0:t0 + tc_n, :])
                    nc.vector.tensor_scalar(
                        dnm_all[:, t0:t0 + tc_n, :],
                        dnm_all[:, t0:t0 + tc_n, :],
                        1e-16, None, mybir.AluOpType.add)
                    nc.vector.reciprocal(rec_all[:, t0:t0 + tc_n, :],
                                         dnm_all[:, t0:t0 + tc_n, :])
                    lt = sb.tile([128, TCMAX, HC], F32, tag="lt", name="lt")
                    nc.vector.tensor_tensor(
                        lt[:, :tc_n, :].rearrange("p t (h c) -> p t h c", h=4),
                        cur[:, t0:t0 + tc_n, 0:128]
                        .rearrange("p t (h c) -> p t h c", h=4),
                        sle[:, t0:t0 + tc_n, :].unsqueeze(3)
                        .to_broadcast([128, tc_n, 4, C]),
                        mybir.AluOpType.mult)
                    nc.vector.tensor_add(lt[:, :tc_n, :], lt[:, :tc_n, :],
                                         aggp[:, :tc_n, :])
                    nc.vector.tensor_tensor(
                        lt[:, :tc_n, :].rearrange("p t (h c) -> p t h c", h=4),
                        lt[:, :tc_n, :].rearrange("p t (h c) -> p t h c", h=4),
                        rec_all[:, t0:t0 + tc_n, :].unsqueeze(3)
                        .to_broadcast([128, tc_n, 4, C]),
                        mybir.AluOpType.mult)
                    nc.vector.tensor_add(
                        lt[:, :tc_n, :], lt[:, :tc_n, :],
                        btf[l][:].unsqueeze(1).to_broadcast([128, tc_n, HC]))
                    nc.vector.tensor_scalar(
                        hbuf[:, t0:t0 + tc_n, :], lt[:, :tc_n, :],
                        0.0, None, mybir.AluOpType.max)

                if l < 3:
                    combN = np2.tile([128, NT, ROWB], F8, tag="comb")
                    scsdN = np2.tile([128, NT, 8], F32, tag="scsd")
                    combined[l % 2] = combN
                    sc_sd[l % 2] = scsdN
                    node_phase(l + 1, lambda t: hbuf[:, t, :], combN, scsdN)
                else:
                    pmat = np1.tile([128, NT, B], BF16, tag="pmat",
                                    name="pmat")
                    nc.sync.dma_start(pmat[:], pmat_in.ap())
                    pl = psB.tile([HC, B], F32, tag="ps1")
                    for t in range(NT):
                        nc.tensor.matmul(pl[:], lhsT=hbuf[:, t, :],
                                         rhs=pmat[:, t, :],
                                         start=(t == 0), stop=(t == NT - 1))
                    pls = sb.tile([HC, B], F32, tag="pls")
                    nc.vector.tensor_copy(pls[:], pl[:])
                    nc.sync.dma_start(pool_in.ap(), pls[:])
                    nc.gpsimd.collective_compute(
                        "AllReduce", mybir.AluOpType.add, replica_groups=rg,
                        ins=[pool_in.ap()], outs=[pool_sh.ap()])
                    plr = sb.tile([HC, B], F32, tag="plr")
                    nc.sync.dma_start(plr[:], pool_sh.ap())
                    zt = psC.tile([A, B], F32, tag="ps2")
                    nc.tensor.matmul(zt[:], lhsT=wl[:], rhs=plr[:],
                                     start=True, stop=True)
                    ot = sb.tile([A, B], F32, tag="ot")
                    nc.scalar.activation(
                        ot[:], zt[:], mybir.ActivationFunctionType.Tanh,
                        bias=blv[:])
                    nc.sync.dma_start(out_t.ap(), ot[:])
    nc.compile()
    return nc


# ================================================================== entry
_CACHE = {}


def _get_nc(layout):
    key = (layout["WTOT"],
           tuple((tuple(ch["tiles"]), ch["RA"], ch["RB"]) for ch in layout["chunks"]))
    if key not in _CACHE:
        _CACHE[key] = build(layout)
    return _CACHE[key]


def kernel(**inputs):
    in_maps, layout = _prep(inputs)
    nc = _get_nc(layout)
    from concourse import bass2jax
    results = bass2jax.run_bass_via_pjrt(nc, in_maps, n_cores=NCORE)
    return np.ascontiguousarray(np.asarray(results[0]["out"], np.float32).T)
